# revision 8
# baseline (speedup 1.0000x reference)
"""Causal attention + output projection on 8 Trainium2 NeuronCores.

Problem (hardcoded): B=2, H=12, T=2048, D=64, DIM=768, fp32.

Sharding: 24 (b, h) pairs -> 3 heads per core; cores 0-3 take b=0,
cores 4-7 take b=1.  Each core computes attention for its 3 heads plus
the partial output projection  sum_h y_h @ W[h*64:(h+1)*64, :]  as a
(T, DIM) fp16 partial; the host sums the 4 partials per batch.  No
collectives.

Device-side layout is fully transposed ([s, q]) so no on-chip
transposes are needed:
  - host feeds qk = [qT/sqrt(D); kT] stacked on 128 partitions (bf16)
  - host feeds biasT = bias^T with the causal mask pre-added
    (-1e4 on s > q) in bf16, packed per (q-chunk j, head) region
  - v is fed augmented with 64 ones-columns (bf16) so a single PV
    matmul yields both y^T (rows 0:64) and the softmax denominators
    replicated across rows 64:128.

All matmuls are bf16 (1 PE cycle/row vs 4 for fp32); PSUM accumulation
stays fp32.  The loop is q-chunk-major (j outer, head inner) so the
output projection and its DMA for chunk j overlap the attention of
chunk j+1.  Per (j, head, group of 2 s-tiles):
  PSUM[s=128, q<=512] <- kT-tile.T @ qT-chunk  (causally trimmed)
  PSUM               += identity @ biasT-tile  (bias copy, trimmed)
  SBUF P = exp(PSUM)   (bf16 out; one ACT instruction per PSUM bank)
  PSUM_y[128, 512]   += vaug-tile.T @ P-slice  (accumulated over s)
then  rec = 1/sums  (DVE partition-realigning read 64:128 -> 0:64),
      yTj[h rows] = y_un * rec  (heads 0,1 stacked on 128 partitions
so the projection contracts 128 rows per matmul).
Projection per t-block: psp = yTj.T @ [W_h0; W_h1] + yTj2.T @ W_h2,
copied to fp16 and DMA'd per chunk.

Bias DMAs are issued on the otherwise-idle Pool (SWDGE) queue, one DMA
per (j, head) in consumption order; qk/va/W loads and output stores go
on the SP queue.
"""

import math

import numpy as np
import ml_dtypes

B, H, T, D = 2, 12, 2048, 64
DIM = H * D
NCORES = 8
HPC = 3           # heads per core
P = 128
QC = 512          # q-chunk width (one PSUM bank of fp32)
NJ = T // QC      # 4 q-chunks
NT = T // P       # 16 s-tiles
GROUP = 2         # s-tiles per PSUM logits group (2 banks)

_PROGRAM = None


def _bias_rows(j):
    """Number of 128-row s-tile rows in the (j, h) bias region."""
    return (j + 1) * 4 * P


def _build_program():
    import concourse.bass as bass
    import concourse.mybir as mybir
    import concourse.tile as tile
    from concourse import bacc
    from contextlib import ExitStack

    dt = mybir.dt
    f32 = dt.float32
    f16 = dt.float16
    bf16 = dt.bfloat16
    EXP = mybir.ActivationFunctionType.Exp
    ds = bass.ds

    nc = bacc.Bacc("TRN2", num_devices=NCORES)
    # per-head [qT | kT] slab (partitions 0:64) and [v|ones] slab, bf16
    qk = nc.declare_dram_parameter("qk", [HPC * D, 2 * T], bf16, isOutput=False)
    va = nc.declare_dram_parameter("va", [HPC * P, T], bf16, isOutput=False)
    # bias regions packed j-major then head: region (j,h) is the
    # [s=0:(j+1)*512, q=j*512:(j+1)*512] block of head h's biasT,
    # flattened s-tile-major
    NBROW = sum(_bias_rows(j) for j in range(NJ)) * HPC
    biasT = nc.declare_dram_parameter("biasT", [NBROW, QC], bf16, isOutput=False)
    # W stacked for head pairing: cols 0:768 = [W_h0; W_h1] (128 rows),
    # cols 768:1536 rows 0:64 = W_h2
    wproj = nc.declare_dram_parameter("wproj", [P, 2 * DIM], bf16, isOutput=False)
    out = nc.declare_dram_parameter("out", [T, DIM], f16, isOutput=True)

    with tile.TileContext(nc) as tc, ExitStack() as ctx:
        from concourse.masks import make_identity

        const_pool = ctx.enter_context(tc.tile_pool(name="const", bufs=1))
        id_t = const_pool.tile([P, P], bf16)
        make_identity(nc, id_t[:])  # gpsimd memset+affine_select: no DMA lane

        # persistent per-head loads (SP queue, issued up front)
        qk_pool = ctx.enter_context(tc.tile_pool(name="qk", bufs=HPC))
        va_pool = ctx.enter_context(tc.tile_pool(name="va", bufs=HPC))
        qk_ts = []
        va_ts = []
        for h in range(HPC):
            t_ = qk_pool.tile([D, 2 * T], bf16)
            nc.sync.dma_start(t_[:], qk[ds(h * D, D), :])
            qk_ts.append(t_)
        for h in range(HPC):
            t_ = va_pool.tile([P, T], bf16)
            nc.sync.dma_start(t_[:], va[ds(h * P, P), :])
            va_ts.append(t_)
        w_pool = ctx.enter_context(tc.tile_pool(name="w", bufs=1))
        w_all = w_pool.tile([P, 2 * DIM], bf16)
        nc.sync.dma_start(w_all[:], wproj[:])
        w01 = w_all[:, 0:DIM]
        w2 = w_all[0:D, DIM : 2 * DIM]

        # bias: one DMA per (j, h) on the Pool SWDGE queue, fixed-size
        # slots sized for j=3 (16 s-tiles)
        bias_pool = ctx.enter_context(tc.tile_pool(name="bias", bufs=4))
        brow0 = []  # DRAM row offset of region (j, h)
        off = 0
        for j in range(NJ):
            for h in range(HPC):
                brow0.append(off)
                off += _bias_rows(j)

        def load_bias(j, h):
            ntile = (j + 1) * 4
            b_t = bias_pool.tile([P, NJ * 4 * QC], bf16)
            nc.gpsimd.dma_start(
                b_t[:, 0 : ntile * QC].rearrange("p (a q) -> p a q", a=ntile),
                biasT[ds(brow0[j * HPC + h], ntile * P), :].rearrange(
                    "(a p) q -> p a q", p=P
                ),
            )
            return b_t

        bias_ts = {}
        # prefetch the first 4 regions before compute starts
        pre = [(0, 0), (0, 1), (0, 2), (1, 0)]
        for j, h in pre:
            bias_ts[(j, h)] = load_bias(j, h)

        with (
            tc.tile_pool(name="pexp", bufs=2) as pexp_pool,
            tc.tile_pool(name="rec", bufs=2) as rec_pool,
            tc.tile_pool(name="yt01", bufs=2) as yt01_pool,
            tc.tile_pool(name="yt2", bufs=2) as yt2_pool,
            tc.tile_pool(name="ob", bufs=2) as ob_pool,
            tc.tile_pool(name="psl", bufs=1, space="PSUM") as psl_pool,
            tc.tile_pool(name="psy", bufs=2, space="PSUM") as psy_pool,
            tc.tile_pool(name="psp", bufs=2, space="PSUM") as psp_pool,
        ):
            order = [(j, h) for j in range(NJ) for h in range(HPC)]
            for j in range(NJ):
                yt01_t = yt01_pool.tile([P, QC], bf16)
                yt2_t = yt2_pool.tile([D, QC], bf16)
                for h in range(HPC):
                    b_t = bias_ts.pop((j, h))
                    # issue the prefetch for the region 4 slots ahead
                    nxt = order.index((j, h)) + 4
                    if nxt < len(order):
                        bias_ts[order[nxt]] = load_bias(*order[nxt])
                    qT_t = qk_ts[h][:, 0:T]
                    kT_t = qk_ts[h][:, T : 2 * T]
                    va_t = va_ts[h]
                    psy_t = psy_pool.tile([P, QC], f32)
                    ngrp = (j + 1) * 4 // GROUP
                    for g in range(ngrp):
                        psl_t = psl_pool.tile([P, GROUP * QC], f32)
                        pe_t = pexp_pool.tile([P, GROUP * QC], bf16)
                        c0s = []
                        for t in range(GROUP):
                            i = g * GROUP + t
                            c0 = max(0, P * i - QC * j)
                            c0s.append(c0)
                            # QK first (can run before the bias arrives)
                            nc.tensor.matmul(
                                psl_t[:, t * QC + c0 : (t + 1) * QC],
                                lhsT=kT_t[:, i * P : (i + 1) * P],
                                rhs=qT_t[:, j * QC + c0 : (j + 1) * QC],
                                start=True,
                                stop=False,
                            )
                            # bias lands on top (identity copy, trimmed)
                            nc.tensor.matmul(
                                psl_t[:, t * QC + c0 : (t + 1) * QC],
                                lhsT=id_t[:],
                                rhs=b_t[:, i * QC + c0 : (i + 1) * QC],
                                start=False,
                                stop=True,
                            )
                        # per-bank exp: subtile release lets the next
                        # group's matmuls re-enter each PSUM bank as soon
                        # as its slice is drained
                        for t in range(GROUP):
                            c0 = c0s[t]
                            nc.scalar.activation(
                                pe_t[:, t * QC + c0 : (t + 1) * QC],
                                psl_t[:, t * QC + c0 : (t + 1) * QC],
                                EXP,
                            )
                        for t in range(GROUP):
                            i = g * GROUP + t
                            c0 = c0s[t]
                            nc.tensor.matmul(
                                psy_t[:, c0:QC],
                                lhsT=va_t[:, i * P : (i + 1) * P],
                                rhs=pe_t[:, t * QC + c0 : (t + 1) * QC],
                                start=(i == 0),
                                stop=(i == 4 * j + 3),
                            )
                    # rows 64:128 of psy hold the softmax denominators
                    # (replicated); realign to partitions 0:64 via the DVE
                    # output crossbar while taking the reciprocal.
                    rec_t = rec_pool.tile([D, QC], f32)
                    nc.vector.reciprocal(rec_t[:], psy_t[D : 2 * D, :])
                    if h < 2:
                        dst = yt01_t[h * D : (h + 1) * D, :]
                    else:
                        dst = yt2_t[:]
                    nc.vector.tensor_mul(dst, psy_t[0:D, :], rec_t[:])
                # projection for chunk j: 4 t-blocks of 128
                ob_t = ob_pool.tile([P, 4 * DIM], f16)
                for tb in range(4):
                    psp_t = psp_pool.tile([P, DIM], f32)
                    for o0, ow in ((0, QC), (QC, DIM - QC)):
                        nc.tensor.matmul(
                            psp_t[:, o0 : o0 + ow],
                            lhsT=yt01_t[:, tb * P : (tb + 1) * P],
                            rhs=w01[:, o0 : o0 + ow],
                            start=True,
                            stop=False,
                        )
                        nc.tensor.matmul(
                            psp_t[:, o0 : o0 + ow],
                            lhsT=yt2_t[:, tb * P : (tb + 1) * P],
                            rhs=w2[:, o0 : o0 + ow],
                            start=False,
                            stop=True,
                        )
                    nc.vector.tensor_copy(
                        ob_t[:, tb * DIM : (tb + 1) * DIM], psp_t[:]
                    )
                nc.sync.dma_start(
                    out[ds(j * QC, QC), :].rearrange("(a p) o -> p a o", p=P),
                    ob_t[:].rearrange("p (a o) -> p a o", a=4),
                )

    nc.finalize()
    return nc


def _get_program():
    global _PROGRAM
    if _PROGRAM is None:
        _PROGRAM = _build_program()
    return _PROGRAM


def make_in_maps(q, k, v, attn_bias, W_proj):
    """Host-side sharding/layout prep: one input map per core."""
    q = np.asarray(q, dtype=np.float32)
    k = np.asarray(k, dtype=np.float32)
    v = np.asarray(v, dtype=np.float32)
    attn_bias = np.asarray(attn_bias, dtype=np.float32)
    W_proj = np.asarray(W_proj, dtype=np.float32)

    scale = 1.0 / math.sqrt(D)
    # causal mask in transposed [s, q] coords: masked where s > q
    smask = (np.arange(T)[:, None] > np.arange(T)[None, :]).astype(np.float32)
    smask *= -10000.0
    w_heads = W_proj.reshape(H, D, DIM)

    in_maps = []
    for c in range(NCORES):
        b = c // 4
        h0 = HPC * (c % 4)
        hs = slice(h0, h0 + HPC)
        qk = np.zeros((HPC, D, 2 * T), dtype=ml_dtypes.bfloat16)
        qk[:, :, :T] = q[b, hs].transpose(0, 2, 1) * scale
        qk[:, :, T:] = k[b, hs].transpose(0, 2, 1)
        va = np.zeros((HPC, P, NT, P), dtype=ml_dtypes.bfloat16)
        va[:, :, :, :D] = v[b, hs].reshape(HPC, NT, P, D).transpose(0, 2, 1, 3)
        va[:, :, :, D:] = 1.0
        biasT = attn_bias[b, hs].transpose(0, 2, 1) + smask[None]
        biasT = biasT.astype(ml_dtypes.bfloat16)
        # region (j, h): s rows [0:(j+1)*512), q cols [j*512:(j+1)*512)
        regions = []
        for j in range(NJ):
            for h in range(HPC):
                regions.append(
                    biasT[h, 0 : (j + 1) * QC, j * QC : (j + 1) * QC]
                )
        bias_pack = np.ascontiguousarray(np.concatenate(regions, axis=0))
        wp = np.zeros((P, 2 * DIM), dtype=ml_dtypes.bfloat16)
        wp[:D, :DIM] = w_heads[h0]
        wp[D:, :DIM] = w_heads[h0 + 1]
        wp[:D, DIM:] = w_heads[h0 + 2]
        in_maps.append(
            {
                "qk": qk.reshape(HPC * D, 2 * T),
                "va": va.reshape(HPC * P, T),
                "biasT": bias_pack,
                "wproj": wp,
            }
        )
    return in_maps


def assemble_output(results):
    """Sum the 4 per-core fp16 partial projections for each batch."""
    out = np.zeros((B, T, DIM), dtype=np.float32)
    for c in range(NCORES):
        out[c // 4] += results[c]["out"].astype(np.float32)
    return out


def kernel(q, k, v, attn_bias, W_proj):
    from concourse.bass_utils import run_bass_kernel_spmd

    nc = _get_program()
    in_maps = make_in_maps(q, k, v, attn_bias, W_proj)
    res = run_bass_kernel_spmd(nc, in_maps, list(range(NCORES)))
    return assemble_output(res.results)


# revision 12
# speedup vs baseline: 1.4779x; 1.4779x over previous
"""Causal attention + output projection on 8 Trainium2 NeuronCores.

Problem (hardcoded): B=2, H=12, T=2048, D=64, DIM=768, fp32.

Sharding: 24 (b, h) pairs -> 3 heads per core; cores 0-3 take b=0,
cores 4-7 take b=1.  Each core computes attention for its 3 heads plus
the partial output projection  sum_h y_h @ W[h*64:(h+1)*64, :]  as a
(T, DIM) fp16 partial; the host sums the 4 partials per batch.  No
collectives.

Device-side layout is fully transposed ([s, q]) so no on-chip
transposes are needed:
  - host feeds qk = [qT/sqrt(D); kT] stacked on 128 partitions (bf16)
  - host feeds biasT = bias^T with the causal mask pre-added
    (-1e4 on s > q) in bf16, packed per (q-chunk j, head) region
  - v is fed augmented with 64 ones-columns (bf16) so a single PV
    matmul yields both y^T (rows 0:64) and the softmax denominators
    replicated across rows 64:128.

All matmuls are bf16 (1 PE cycle/row vs 4 for fp32); PSUM accumulation
stays fp32.  The loop is q-chunk-major (j outer, head inner) so the
output projection and its DMA for chunk j overlap the attention of
chunk j+1.  Per (j, head, group of 2 s-tiles):
  PSUM[s=128, q<=512] <- kT-tile.T @ qT-chunk  (causally trimmed)
  PSUM               += identity @ biasT-tile  (bias copy, trimmed)
  SBUF P = exp(PSUM)   (bf16 out; one ACT instruction per PSUM bank)
  PSUM_y[128, 512]   += vaug-tile.T @ P-slice  (accumulated over s)
then  rec = 1/sums  (DVE partition-realigning read 64:128 -> 0:64),
      yTj[h rows] = y_un * rec  (heads 0,1 stacked on 128 partitions
so the projection contracts 128 rows per matmul).
Projection per t-block: psp = yTj.T @ [W_h0; W_h1] + yTj2.T @ W_h2,
copied to fp16 and DMA'd per chunk.

Bias DMAs are issued on the otherwise-idle Pool (SWDGE) queue, one DMA
per (j, head) in consumption order; qk/va/W loads and output stores go
on the SP queue.
"""

import math

import numpy as np
import ml_dtypes

B, H, T, D = 2, 12, 2048, 64
DIM = H * D
NCORES = 8
HPC = 3           # heads per core
P = 128
QC = 512          # q-chunk width (one PSUM bank of fp32)
NJ = T // QC      # 4 q-chunks
NT = T // P       # 16 s-tiles
GROUP = 2         # s-tiles per PSUM logits group (2 banks)

_PROGRAM = None


def _bias_rows(j):
    """Number of 128-row s-tile rows in the (j, h) bias region."""
    return (j + 1) * 4 * P


def _build_program():
    import concourse.bass as bass
    import concourse.mybir as mybir
    import concourse.tile as tile
    from concourse import bacc
    from contextlib import ExitStack

    dt = mybir.dt
    f32 = dt.float32
    f16 = dt.float16
    bf16 = dt.bfloat16
    EXP = mybir.ActivationFunctionType.Exp
    ds = bass.ds

    nc = bacc.Bacc("TRN2", num_devices=NCORES)
    # per-head [qT | kT] slab (partitions 0:64) and [v|ones] slab, bf16
    qk = nc.declare_dram_parameter("qk", [HPC * D, 2 * T], bf16, isOutput=False)
    va = nc.declare_dram_parameter("va", [HPC * P, T], bf16, isOutput=False)
    # bias regions packed j-major then head: region (j,h) is the
    # [s=0:(j+1)*512, q=j*512:(j+1)*512] block of head h's biasT,
    # flattened s-tile-major
    NBROW = sum(_bias_rows(j) for j in range(NJ)) * HPC
    biasT = nc.declare_dram_parameter("biasT", [NBROW, QC], bf16, isOutput=False)
    # W stacked for head pairing: cols 0:768 = [W_h0; W_h1] (128 rows),
    # cols 768:1536 rows 0:64 = W_h2
    wproj = nc.declare_dram_parameter("wproj", [P, 2 * DIM], bf16, isOutput=False)
    out = nc.declare_dram_parameter("out", [T, DIM], f16, isOutput=True)

    with tile.TileContext(nc) as tc, ExitStack() as ctx:
        from concourse.masks import make_identity

        const_pool = ctx.enter_context(tc.tile_pool(name="const", bufs=1))
        id_t = const_pool.tile([P, P], bf16)
        make_identity(nc, id_t[:])  # gpsimd memset+affine_select: no DMA lane

        # persistent per-head loads (SP queue, issued up front)
        qk_pool = ctx.enter_context(tc.tile_pool(name="qk", bufs=HPC))
        va_pool = ctx.enter_context(tc.tile_pool(name="va", bufs=HPC))
        qk_ts = []
        va_ts = []
        for h in range(HPC):
            t_ = qk_pool.tile([D, 2 * T], bf16)
            nc.sync.dma_start(t_[:], qk[ds(h * D, D), :])
            qk_ts.append(t_)
        for h in range(HPC):
            t_ = va_pool.tile([P, T], bf16)
            nc.sync.dma_start(t_[:], va[ds(h * P, P), :])
            va_ts.append(t_)
        w_pool = ctx.enter_context(tc.tile_pool(name="w", bufs=1))
        w_all = w_pool.tile([P, 2 * DIM], bf16)
        nc.sync.dma_start(w_all[:], wproj[:])
        w01 = w_all[:, 0:DIM]
        w2 = w_all[0:D, DIM : 2 * DIM]

        # bias: one DMA per (j, h), fixed-size slots sized for j=3
        # (16 s-tiles).  j=0 regions go on the SP queue interleaved with
        # the qk/va loads so the first groups can start early; the rest
        # go on the otherwise-idle Pool (SWDGE) queue.
        bias_pool = ctx.enter_context(tc.tile_pool(name="bias", bufs=4))
        brow0 = []  # DRAM row offset of region (j, h)
        off = 0
        for j in range(NJ):
            for h in range(HPC):
                brow0.append(off)
                off += _bias_rows(j)

        def load_bias(j, h, eng):
            ntile = (j + 1) * 4
            b_t = bias_pool.tile([P, NJ * 4 * QC], bf16)
            eng.dma_start(
                b_t[:, 0 : ntile * QC].rearrange("p (a q) -> p a q", a=ntile),
                biasT[ds(brow0[j * HPC + h], ntile * P), :].rearrange(
                    "(a p) q -> p a q", p=P
                ),
            )
            return b_t

        bias_ts = {}
        for h in range(HPC):
            bias_ts[(0, h)] = load_bias(0, h, nc.sync)
        bias_ts[(1, 0)] = load_bias(1, 0, nc.gpsimd)

        with (
            tc.tile_pool(name="pexp", bufs=4) as pexp_pool,
            tc.tile_pool(name="rec", bufs=2) as rec_pool,
            tc.tile_pool(name="yt01", bufs=2) as yt01_pool,
            tc.tile_pool(name="yt2", bufs=2) as yt2_pool,
            tc.tile_pool(name="ob", bufs=2) as ob_pool,
            tc.tile_pool(name="psl", bufs=3, space="PSUM") as psl_pool,
            tc.tile_pool(name="psy", bufs=1, space="PSUM") as psy_pool,
            tc.tile_pool(name="psp", bufs=2, space="PSUM") as psp_pool,
        ):
            yt_box = [None, None]

            def emit_drain(j, h, psy_t):
                # rows 64:128 of psy hold the softmax denominators
                # (replicated); realign to partitions 0:64 via the DVE
                # output crossbar while taking the reciprocal.
                if h == 0:
                    yt_box[0] = yt01_pool.tile([P, QC], bf16, name="yt01_t")
                    yt_box[1] = yt2_pool.tile([D, QC], bf16, name="yt2_t")
                yt01_t, yt2_t = yt_box
                rec_t = rec_pool.tile([D, QC], f32)
                nc.vector.reciprocal(rec_t[:], psy_t[D : 2 * D, :])
                if h < 2:
                    dst = yt01_t[h * D : (h + 1) * D, :]
                else:
                    dst = yt2_t[:]
                nc.vector.tensor_mul(dst, psy_t[0:D, :], rec_t[:])
                if h < 2:
                    return
                emit_proj(j, yt01_t, yt2_t)

            def emit_proj(j, yt01_t, yt2_t):
                # projection for chunk j: 4 t-blocks of 128, one output
                # DMA per t-block to keep the drain tail short
                for tb in range(4):
                    psp_t = psp_pool.tile([P, DIM], f32)
                    for o0, ow in ((0, QC), (QC, DIM - QC)):
                        nc.tensor.matmul(
                            psp_t[:, o0 : o0 + ow],
                            lhsT=yt01_t[:, tb * P : (tb + 1) * P],
                            rhs=w01[:, o0 : o0 + ow],
                            start=True,
                            stop=False,
                        )
                        nc.tensor.matmul(
                            psp_t[:, o0 : o0 + ow],
                            lhsT=yt2_t[:, tb * P : (tb + 1) * P],
                            rhs=w2[:, o0 : o0 + ow],
                            start=False,
                            stop=True,
                        )
                    ob_t = ob_pool.tile([P, DIM], f16)
                    nc.vector.tensor_copy(ob_t[:], psp_t[:])
                    nc.sync.dma_start(
                        out[ds(j * QC + tb * P, P), :], ob_t[:]
                    )

            order = [(j, h) for j in range(NJ) for h in range(HPC)]
            # software pipeline: the PV of logits-tile i is emitted LAG
            # tiles after its QK/copy/exp, so the in-order PE queue always
            # has matmul work while the ACT engine runs the exps, and the
            # chunk-j projection trails one tile further still
            from collections import deque

            LAG = 2
            queue = deque()
            for j, h in order:
                b_t = bias_ts.pop((j, h))
                nxt = order.index((j, h)) + 4
                if nxt < len(order):
                    bias_ts[order[nxt]] = load_bias(*order[nxt], nc.gpsimd)
                qT_t = qk_ts[h][:, 0:T]
                kT_t = qk_ts[h][:, T : 2 * T]
                va_t = va_ts[h]
                psy_box = [None]
                ntile = (j + 1) * 4
                for i in range(ntile):
                    c0 = max(0, P * i - QC * j)
                    psl_t = psl_pool.tile([P, QC], f32)
                    # QK first (can run before the bias arrives)
                    nc.tensor.matmul(
                        psl_t[:, c0:QC],
                        lhsT=kT_t[:, i * P : (i + 1) * P],
                        rhs=qT_t[:, j * QC + c0 : (j + 1) * QC],
                        start=True,
                        stop=False,
                    )
                    # bias lands on top (identity copy, trimmed)
                    nc.tensor.matmul(
                        psl_t[:, c0:QC],
                        lhsT=id_t[:],
                        rhs=b_t[:, i * QC + c0 : (i + 1) * QC],
                        start=False,
                        stop=True,
                    )
                    pe_t = pexp_pool.tile([P, QC], bf16)
                    nc.scalar.activation(
                        pe_t[:, c0:QC], psl_t[:, c0:QC], EXP
                    )

                    def mk_pv(j, h, i, c0, pe_t, psy_box, va_t):
                        def emit():
                            if i == 0:
                                psy_box[0] = psy_pool.tile(
                                    [P, QC], f32, name="psy_t"
                                )
                            psy_t = psy_box[0]
                            nc.tensor.matmul(
                                psy_t[:, c0:QC],
                                lhsT=va_t[:, i * P : (i + 1) * P],
                                rhs=pe_t[:, c0:QC],
                                start=(i == 0),
                                stop=(i == 4 * j + 3),
                            )
                            if i == 4 * j + 3:
                                emit_drain(j, h, psy_t)
                        return emit

                    queue.append(mk_pv(j, h, i, c0, pe_t, psy_box, va_t))
                    while len(queue) > LAG:
                        queue.popleft()()
            while queue:
                queue.popleft()()

    nc.finalize()
    return nc


def _get_program():
    global _PROGRAM
    if _PROGRAM is None:
        _PROGRAM = _build_program()
    return _PROGRAM


def make_in_maps(q, k, v, attn_bias, W_proj):
    """Host-side sharding/layout prep: one input map per core."""
    q = np.asarray(q, dtype=np.float32)
    k = np.asarray(k, dtype=np.float32)
    v = np.asarray(v, dtype=np.float32)
    attn_bias = np.asarray(attn_bias, dtype=np.float32)
    W_proj = np.asarray(W_proj, dtype=np.float32)

    scale = 1.0 / math.sqrt(D)
    # causal mask in transposed [s, q] coords: masked where s > q
    smask = (np.arange(T)[:, None] > np.arange(T)[None, :]).astype(np.float32)
    smask *= -10000.0
    w_heads = W_proj.reshape(H, D, DIM)

    in_maps = []
    for c in range(NCORES):
        b = c // 4
        h0 = HPC * (c % 4)
        hs = slice(h0, h0 + HPC)
        qk = np.zeros((HPC, D, 2 * T), dtype=ml_dtypes.bfloat16)
        qk[:, :, :T] = q[b, hs].transpose(0, 2, 1) * scale
        qk[:, :, T:] = k[b, hs].transpose(0, 2, 1)
        va = np.zeros((HPC, P, NT, P), dtype=ml_dtypes.bfloat16)
        va[:, :, :, :D] = v[b, hs].reshape(HPC, NT, P, D).transpose(0, 2, 1, 3)
        va[:, :, :, D:] = 1.0
        biasT = attn_bias[b, hs].transpose(0, 2, 1) + smask[None]
        biasT = biasT.astype(ml_dtypes.bfloat16)
        # region (j, h): s rows [0:(j+1)*512), q cols [j*512:(j+1)*512)
        regions = []
        for j in range(NJ):
            for h in range(HPC):
                regions.append(
                    biasT[h, 0 : (j + 1) * QC, j * QC : (j + 1) * QC]
                )
        bias_pack = np.ascontiguousarray(np.concatenate(regions, axis=0))
        wp = np.zeros((P, 2 * DIM), dtype=ml_dtypes.bfloat16)
        wp[:D, :DIM] = w_heads[h0]
        wp[D:, :DIM] = w_heads[h0 + 1]
        wp[:D, DIM:] = w_heads[h0 + 2]
        in_maps.append(
            {
                "qk": qk.reshape(HPC * D, 2 * T),
                "va": va.reshape(HPC * P, T),
                "biasT": bias_pack,
                "wproj": wp,
            }
        )
    return in_maps


def assemble_output(results):
    """Sum the 4 per-core fp16 partial projections for each batch."""
    out = np.zeros((B, T, DIM), dtype=np.float32)
    for c in range(NCORES):
        out[c // 4] += results[c]["out"].astype(np.float32)
    return out


def kernel(q, k, v, attn_bias, W_proj):
    from concourse.bass_utils import run_bass_kernel_spmd

    nc = _get_program()
    in_maps = make_in_maps(q, k, v, attn_bias, W_proj)
    res = run_bass_kernel_spmd(nc, in_maps, list(range(NCORES)))
    return assemble_output(res.results)


# revision 18
# speedup vs baseline: 1.7734x; 1.1999x over previous
"""Causal attention + output projection on 8 Trainium2 NeuronCores.

Problem (hardcoded): B=2, H=12, T=2048, D=64, DIM=768, fp32.

Sharding: 24 (b, h) pairs -> 3 heads per core; cores 0-3 take b=0,
cores 4-7 take b=1.  Each core computes attention for its 3 heads plus
the partial output projection  sum_h y_h @ W[h*64:(h+1)*64, :]  as a
(T, DIM) fp16 partial; the host sums the 4 partials per batch.  No
collectives.

Device-side layout is fully transposed ([s, q]) so no on-chip
transposes are needed:
  - host feeds qk = [qT/sqrt(D); kT] stacked on 128 partitions (bf16)
  - host feeds biasT = bias^T with the causal mask pre-added
    (-1e4 on s > q) in bf16, packed per (q-chunk j, head) region
  - v is fed augmented with 64 ones-columns (bf16) so a single PV
    matmul yields both y^T (rows 0:64) and the softmax denominators
    replicated across rows 64:128.

All matmuls are bf16 (1 PE cycle/row vs 4 for fp32); PSUM accumulation
stays fp32.  The loop is q-chunk-major (j outer, head inner) so the
output projection and its DMA for chunk j overlap the attention of
chunk j+1.  Per (j, head, group of 2 s-tiles):
  PSUM[s=128, q<=512] <- kT-tile.T @ qT-chunk  (causally trimmed)
  PSUM               += identity @ biasT-tile  (bias copy, trimmed)
  SBUF P = exp(PSUM)   (bf16 out; one ACT instruction per PSUM bank)
  PSUM_y[128, 512]   += vaug-tile.T @ P-slice  (accumulated over s)
then  rec = 1/sums  (DVE partition-realigning read 64:128 -> 0:64),
      yTj[h rows] = y_un * rec  (heads 0,1 stacked on 128 partitions
so the projection contracts 128 rows per matmul).
Projection per t-block: psp = yTj.T @ [W_h0; W_h1] + yTj2.T @ W_h2,
copied to fp16 and DMA'd per chunk.

Bias DMAs are issued on the otherwise-idle Pool (SWDGE) queue, one DMA
per (j, head) in consumption order; qk/va/W loads and output stores go
on the SP queue.
"""

import math

import numpy as np
import ml_dtypes

B, H, T, D = 2, 12, 2048, 64
DIM = H * D
NCORES = 8
HPC = 3           # heads per core
P = 128
QC = 512          # q-chunk width (one PSUM bank of fp32)
NJ = T // QC      # 4 q-chunks
NT = T // P       # 16 s-tiles
GROUP = 2         # s-tiles per PSUM logits group (2 banks)

_PROGRAM = None


def _bias_rows(j):
    """Number of 128-row s-tile rows in the (j, h) bias region."""
    return (j + 1) * 4 * P


def _build_program():
    import concourse.bass as bass
    import concourse.mybir as mybir
    import concourse.tile as tile
    from concourse import bacc
    from contextlib import ExitStack

    dt = mybir.dt
    f32 = dt.float32
    f16 = dt.float16
    bf16 = dt.bfloat16
    f8 = dt.float8e4
    EXP = mybir.ActivationFunctionType.Exp
    ds = bass.ds

    nc = bacc.Bacc("TRN2", num_devices=NCORES)
    # [qT | kT] slabs, bf16: rows 0:128 = heads 0,1 stacked on 128
    # partitions (full-rate DMA), rows 128:192 = head 2
    qk = nc.declare_dram_parameter("qk", [P + D, 2 * T], bf16, isOutput=False)
    va = nc.declare_dram_parameter("va", [HPC * P, T], bf16, isOutput=False)
    # bias regions packed j-major then head: region (j,h) is the
    # [s=0:(j+1)*512, q=j*512:(j+1)*512] block of head h's biasT,
    # flattened s-tile-major
    NBROW = sum(_bias_rows(j) for j in range(NJ)) * HPC // 2
    biasT = nc.declare_dram_parameter("biasT", [NBROW, 2 * QC], f8, isOutput=False)
    # DoubleRow selector weights: cols 0:256 pick pair slot A, 256:512 slot B
    idpk = nc.declare_dram_parameter("idpk", [P, 4 * P], f8, isOutput=False)
    # W stacked for head pairing: cols 0:768 = [W_h0; W_h1] (128 rows),
    # cols 768:1536 rows 0:64 = W_h2
    wproj = nc.declare_dram_parameter("wproj", [P, 2 * DIM], bf16, isOutput=False)
    out = nc.declare_dram_parameter("out", [T, DIM], f16, isOutput=True)

    with tile.TileContext(nc) as tc, ExitStack() as ctx:
        # bias is streamed in sub-regions of 4 consecutive s-tiles
        # ([128, 4x512] bf16): fine enough that the first compute starts
        # ~4us in and the big j>=1 regions never head-of-line-block the
        # DMA queue.  The first three subs (j=0) go on the SP queue
        # interleaved with the qk/va loads in consumption order; the
        # rest go on the otherwise-idle Pool (SWDGE) queue with a
        # rolling lookahead.
        brow0 = {}  # DRAM row offset of region (j, h)
        off = 0
        for j in (3, 2, 0, 1):
            for h in range(HPC):
                brow0[(j, h)] = off
                off += _bias_rows(j) // 2
        subs = []  # (j, h, k) in consumption order (j descending: the
        # j=3 regions have the thickest compute, best able to hide the
        # DMA stream warming up)
        for j in (3, 2, 0, 1):
            for h in range(HPC):
                for k in range(j + 1):
                    subs.append((j, h, k))

        bias_pool = ctx.enter_context(tc.tile_pool(name="bias", bufs=10))
        bias_ts = {}
        nissued = [0]

        def issue_subs(upto, eng):
            while nissued[0] < min(upto, len(subs)):
                j, h, k = subs[nissued[0]]
                b_t = bias_pool.tile([P, 4 * QC], f8, name="bsub_t")
                eng.dma_start(
                    b_t[:].rearrange("p (a q) -> p a q", a=2),
                    biasT[ds(brow0[(j, h)] + k * 2 * P, 2 * P), :].rearrange(
                        "(a p) q -> p a q", p=P
                    ),
                )
                bias_ts[(j, h, k)] = b_t
                nissued[0] += 1

        # startup loads in consumption order, all on the SP queue.
        # qk slab pieces are separate tiles so the first QK only waits
        # on its own piece: A = qT chunk j=3 + kT s-tiles 0:8 for heads
        # 0,1; B2 = kT s-tiles 8:16; B1 = qT chunks j=0,1,2 (consumed
        # much later); head 2 loads as one 64-partition slab.
        qk_pool = ctx.enter_context(tc.tile_pool(name="qk", bufs=4))
        id8_pool = ctx.enter_context(tc.tile_pool(name="id8", bufs=1))
        id8_t = id8_pool.tile([P, 4 * P], f8)
        nc.sync.dma_start(id8_t[:], idpk[:])
        slabA = qk_pool.tile([P, 3 * QC], bf16, name="slabA")
        nc.sync.dma_start(slabA[:], qk[0:P, 3 * QC : 6 * QC])
        issue_subs(1, nc.sync)
        va_pool = ctx.enter_context(tc.tile_pool(name="va", bufs=HPC))
        va_ts = []

        def load_va(h):
            va_t = va_pool.tile([P, T], bf16, name="va_t")
            nc.sync.dma_start(va_t[:], va[ds(h * P, P), :])
            va_ts.append(va_t)

        load_va(0)
        issue_subs(2, nc.sync)
        slabB2 = qk_pool.tile([P, 2 * QC], bf16, name="slabB2")
        nc.sync.dma_start(slabB2[:], qk[0:P, 6 * QC : 8 * QC])
        issue_subs(3, nc.sync)
        load_va(1)
        issue_subs(4, nc.sync)
        slab2 = qk_pool.tile([D, 2 * T], bf16, name="slab2")
        nc.sync.dma_start(slab2[:], qk[ds(P, D), :])
        load_va(2)
        slabB1 = qk_pool.tile([P, 3 * QC], bf16, name="slabB1")
        nc.sync.dma_start(slabB1[:], qk[0:P, 0 : 3 * QC])

        def qT_ap(h, j):
            if h == 2:
                return slab2[:, j * QC : (j + 1) * QC]
            r = slice(h * D, (h + 1) * D)
            if j == 3:
                return slabA[r, 0:QC]
            return slabB1[r, j * QC : (j + 1) * QC]

        def kT_ap(h, i):
            if h == 2:
                return slab2[:, 4 * QC + i * P : 4 * QC + (i + 1) * P]
            r = slice(h * D, (h + 1) * D)
            if i < 8:
                return slabA[r, QC + i * P : QC + (i + 1) * P]
            return slabB2[r, (i - 8) * P : (i - 7) * P]

        w_pool = ctx.enter_context(tc.tile_pool(name="w", bufs=1))
        w_all = w_pool.tile([P, 2 * DIM], bf16)
        nc.sync.dma_start(w_all[:], wproj[:])
        w01 = w_all[:, 0:DIM]
        w2 = w_all[0:D, DIM : 2 * DIM]

        with (
            tc.tile_pool(name="pexp", bufs=4) as pexp_pool,
            tc.tile_pool(name="rec", bufs=2) as rec_pool,
            tc.tile_pool(name="yt01", bufs=2) as yt01_pool,
            tc.tile_pool(name="yt2", bufs=2) as yt2_pool,
            tc.tile_pool(name="ob", bufs=2) as ob_pool,
            tc.tile_pool(name="psl", bufs=3, space="PSUM") as psl_pool,
            tc.tile_pool(name="psy", bufs=1, space="PSUM") as psy_pool,
            tc.tile_pool(name="psp", bufs=2, space="PSUM") as psp_pool,
        ):
            yt_box = [None, None]

            def emit_drain(j, h, psy_t):
                # rows 64:128 of psy hold the softmax denominators
                # (replicated); realign to partitions 0:64 via the DVE
                # output crossbar while taking the reciprocal.
                if h == 0:
                    yt_box[0] = yt01_pool.tile([P, QC], bf16, name="yt01_t")
                    yt_box[1] = yt2_pool.tile([D, QC], bf16, name="yt2_t")
                yt01_t, yt2_t = yt_box
                rec_t = rec_pool.tile([D, QC], f32)
                nc.vector.reciprocal(rec_t[:], psy_t[D : 2 * D, :])
                if h < 2:
                    dst = yt01_t[h * D : (h + 1) * D, :]
                else:
                    dst = yt2_t[:]
                nc.vector.tensor_mul(dst, psy_t[0:D, :], rec_t[:])
                if h < 2:
                    return
                emit_proj(j, yt01_t, yt2_t)

            def emit_proj(j, yt01_t, yt2_t):
                # projection for chunk j: 4 t-blocks of 128, one output
                # DMA per t-block to keep the drain tail short
                for tb in range(4):
                    psp_t = psp_pool.tile([P, DIM], f32)
                    for o0, ow in ((0, QC), (QC, DIM - QC)):
                        nc.tensor.matmul(
                            psp_t[:, o0 : o0 + ow],
                            lhsT=yt01_t[:, tb * P : (tb + 1) * P],
                            rhs=w01[:, o0 : o0 + ow],
                            start=True,
                            stop=False,
                        )
                        nc.tensor.matmul(
                            psp_t[:, o0 : o0 + ow],
                            lhsT=yt2_t[:, tb * P : (tb + 1) * P],
                            rhs=w2[:, o0 : o0 + ow],
                            start=False,
                            stop=True,
                        )
                    ob_t = ob_pool.tile([P, DIM], f16)
                    nc.vector.tensor_copy(ob_t[:], psp_t[:])
                    nc.sync.dma_start(
                        out[ds(j * QC + tb * P, P), :], ob_t[:]
                    )

            order = [(j, h) for j in (3, 2, 0, 1) for h in range(HPC)]
            # software pipeline: the PV of logits-tile i is emitted LAG
            # tiles after its QK/copy/exp, so the in-order PE queue always
            # has matmul work while the ACT engine runs the exps, and the
            # chunk-j projection trails one tile further still
            from collections import deque

            LAG = 2
            queue = deque()
            sub0 = {}  # first sub index of region (j, h)
            for n, (j, h, k) in enumerate(subs):
                if k == 0:
                    sub0[(j, h)] = n
            for j, h in order:
                issue_subs(sub0[(j, h)] + (j + 1) + 4, nc.gpsimd)
                qT_t = qT_ap(h, j)
                va_t = va_ts[h]
                psy_box = [None]
                ntile = (j + 1) * 4
                for i in range(ntile):
                    c0 = max(0, P * i - QC * j)
                    psl_t = psl_pool.tile([P, QC], f32)
                    # QK first (can run before the bias arrives)
                    nc.tensor.matmul(
                        psl_t[:, c0:QC],
                        lhsT=kT_ap(h, i),
                        rhs=qT_t[:, c0:QC],
                        start=True,
                        stop=False,
                    )
                    # bias lands on top: fp8 DoubleRow identity copy
                    # (0.5 PE cycles/row); tile pairs are column-interleaved
                    # in SBUF, the selector weights pick one tile per instr
                    pr, sl = (i % 4) // 2, i % 2
                    nc.tensor.matmul(
                        psl_t[:, c0:QC],
                        lhsT=id8_t[
                            :, sl * 2 * P : (sl + 1) * 2 * P
                        ].rearrange("p (two m) -> p two m", two=2),
                        rhs=bias_ts[(j, h, i // 4)][
                            :, pr * 2 * QC : (pr + 1) * 2 * QC
                        ].rearrange("p (two q) -> p two q", two=2)[:, :, c0:],
                        start=False,
                        stop=True,
                        perf_mode=mybir.MatmulPerfMode.DoubleRow,
                    )
                    pe_t = pexp_pool.tile([P, QC], bf16)
                    nc.scalar.activation(
                        pe_t[:, c0:QC], psl_t[:, c0:QC], EXP
                    )

                    def mk_pv(j, h, i, c0, pe_t, psy_box, va_t):
                        def emit():
                            if i == 0:
                                psy_box[0] = psy_pool.tile(
                                    [P, QC], f32, name="psy_t"
                                )
                            psy_t = psy_box[0]
                            nc.tensor.matmul(
                                psy_t[:, c0:QC],
                                lhsT=va_t[:, i * P : (i + 1) * P],
                                rhs=pe_t[:, c0:QC],
                                start=(i == 0),
                                stop=(i == 4 * j + 3),
                            )
                            if i == 4 * j + 3:
                                emit_drain(j, h, psy_t)
                        return emit

                    queue.append(mk_pv(j, h, i, c0, pe_t, psy_box, va_t))
                    while len(queue) > LAG:
                        queue.popleft()()
            while queue:
                queue.popleft()()

    nc.finalize()
    return nc


def _get_program():
    global _PROGRAM
    if _PROGRAM is None:
        _PROGRAM = _build_program()
    return _PROGRAM


def make_in_maps(q, k, v, attn_bias, W_proj):
    """Host-side sharding/layout prep: one input map per core."""
    q = np.asarray(q, dtype=np.float32)
    k = np.asarray(k, dtype=np.float32)
    v = np.asarray(v, dtype=np.float32)
    attn_bias = np.asarray(attn_bias, dtype=np.float32)
    W_proj = np.asarray(W_proj, dtype=np.float32)

    scale = 1.0 / math.sqrt(D)
    # causal mask in transposed [s, q] coords: masked where s > q
    smask = (np.arange(T)[:, None] > np.arange(T)[None, :]).astype(np.float32)
    smask *= -10000.0
    w_heads = W_proj.reshape(H, D, DIM)

    in_maps = []
    for c in range(NCORES):
        b = c // 4
        h0 = HPC * (c % 4)
        hs = slice(h0, h0 + HPC)
        qk = np.zeros((P + D, 2 * T), dtype=ml_dtypes.bfloat16)
        for hh in range(HPC):
            qk[hh * D : (hh + 1) * D, :T] = (
                q[b, h0 + hh].transpose(1, 0) * scale
            )
            qk[hh * D : (hh + 1) * D, T:] = k[b, h0 + hh].transpose(1, 0)
        va = np.zeros((HPC, P, NT, P), dtype=ml_dtypes.bfloat16)
        va[:, :, :, :D] = v[b, hs].reshape(HPC, NT, P, D).transpose(0, 2, 1, 3)
        va[:, :, :, D:] = 1.0
        biasT = attn_bias[b, hs].transpose(0, 2, 1) + smask[None]
        biasT = np.clip(biasT, -240.0, 240.0).astype(ml_dtypes.float8_e4m3)
        # region (j, h): s rows [0:(j+1)*512), q cols [j*512:(j+1)*512),
        # s-tile pairs column-interleaved for the DoubleRow copy
        regions = []
        for j in (3, 2, 0, 1):
            for h in range(HPC):
                r = biasT[h, 0 : (j + 1) * QC, j * QC : (j + 1) * QC]
                pairs = r.reshape((j + 1) * 2, 2, P, QC)
                regions.append(
                    pairs.transpose(0, 2, 1, 3).reshape((j + 1) * 2 * P, 2 * QC)
                )
        bias_pack = np.ascontiguousarray(np.concatenate(regions, axis=0))
        idp = np.zeros((P, 4 * P), dtype=ml_dtypes.float8_e4m3)
        eye = np.eye(P, dtype=np.float32)
        idp[:, 0:P] = eye        # selector A = [I | 0]
        idp[:, 3 * P :] = eye    # selector B = [0 | I]
        wp = np.zeros((P, 2 * DIM), dtype=ml_dtypes.bfloat16)
        wp[:D, :DIM] = w_heads[h0]
        wp[D:, :DIM] = w_heads[h0 + 1]
        wp[:D, DIM:] = w_heads[h0 + 2]
        in_maps.append(
            {
                "qk": qk,
                "va": va.reshape(HPC * P, T),
                "biasT": bias_pack,
                "idpk": idp,
                "wproj": wp,
            }
        )
    return in_maps


def assemble_output(results):
    """Sum the 4 per-core fp16 partial projections for each batch."""
    out = np.zeros((B, T, DIM), dtype=np.float32)
    for c in range(NCORES):
        out[c // 4] += results[c]["out"].astype(np.float32)
    return out


def kernel(q, k, v, attn_bias, W_proj):
    from concourse.bass_utils import run_bass_kernel_spmd

    nc = _get_program()
    in_maps = make_in_maps(q, k, v, attn_bias, W_proj)
    res = run_bass_kernel_spmd(nc, in_maps, list(range(NCORES)))
    return assemble_output(res.results)


# revision 24
# speedup vs baseline: 1.8701x; 1.0545x over previous
"""Causal attention + output projection on 8 Trainium2 NeuronCores.

Problem (hardcoded): B=2, H=12, T=2048, D=64, DIM=768, fp32.

Sharding: 24 (b, h) pairs -> 3 heads per core; cores 0-3 take b=0,
cores 4-7 take b=1.  Each core computes attention for its 3 heads plus
the partial output projection  sum_h y_h @ W[h*64:(h+1)*64, :]  as a
(T, DIM) fp16 partial; the host sums the 4 partials per batch.  No
collectives.

Device-side layout is fully transposed ([s, q]) so no on-chip
transposes are needed:
  - host feeds qk = [qT/sqrt(D); kT] stacked on 128 partitions (bf16)
  - host feeds biasT = bias^T with the causal mask pre-added
    (-1e4 on s > q) in bf16, packed per (q-chunk j, head) region
  - v is fed augmented with 64 ones-columns (bf16) so a single PV
    matmul yields both y^T (rows 0:64) and the softmax denominators
    replicated across rows 64:128.

All matmuls are bf16 (1 PE cycle/row vs 4 for fp32); PSUM accumulation
stays fp32.  The loop is q-chunk-major (j outer, head inner) so the
output projection and its DMA for chunk j overlap the attention of
chunk j+1.  Per (j, head, group of 2 s-tiles):
  PSUM[s=128, q<=512] <- kT-tile.T @ qT-chunk  (causally trimmed)
  PSUM               += identity @ biasT-tile  (bias copy, trimmed)
  SBUF P = exp(PSUM)   (bf16 out; one ACT instruction per PSUM bank)
  PSUM_y[128, 512]   += vaug-tile.T @ P-slice  (accumulated over s)
then  rec = 1/sums  (DVE partition-realigning read 64:128 -> 0:64),
      yTj[h rows] = y_un * rec  (heads 0,1 stacked on 128 partitions
so the projection contracts 128 rows per matmul).
Projection per t-block: psp = yTj.T @ [W_h0; W_h1] + yTj2.T @ W_h2,
copied to fp16 and DMA'd per chunk.

Bias DMAs are issued on the otherwise-idle Pool (SWDGE) queue, one DMA
per (j, head) in consumption order; qk/va/W loads and output stores go
on the SP queue.
"""

import math

import numpy as np
import ml_dtypes

B, H, T, D = 2, 12, 2048, 64
DIM = H * D
NCORES = 8
HPC = 3           # heads per core
P = 128
QC = 512          # q-chunk width (one PSUM bank of fp32)
NJ = T // QC      # 4 q-chunks
NT = T // P       # 16 s-tiles
GROUP = 2         # s-tiles per PSUM logits group (2 banks)

_PROGRAM = None


def _bias_rows(j):
    """Number of 128-row s-tile rows in the (j, h) bias region."""
    return (j + 1) * 4 * P


def _build_program():
    import concourse.bass as bass
    import concourse.mybir as mybir
    import concourse.tile as tile
    from concourse import bacc
    from contextlib import ExitStack

    dt = mybir.dt
    f32 = dt.float32
    f16 = dt.float16
    bf16 = dt.bfloat16
    f8 = dt.float8e4
    EXP = mybir.ActivationFunctionType.Exp
    ds = bass.ds

    nc = bacc.Bacc("TRN2", num_devices=NCORES)
    # [qT | kT] slabs, bf16: rows 0:128 = heads 0,1 stacked on 128
    # partitions (full-rate DMA), rows 128:192 = head 2
    qk = nc.declare_dram_parameter("qk", [P + D, 2 * T], bf16, isOutput=False)
    va = nc.declare_dram_parameter("va", [HPC * P, T], bf16, isOutput=False)
    # bias regions packed j-major then head: region (j,h) is the
    # [s=0:(j+1)*512, q=j*512:(j+1)*512] block of head h's biasT,
    # flattened s-tile-major
    NBROW = sum(_bias_rows(j) for j in range(NJ)) * HPC // 2
    biasT = nc.declare_dram_parameter("biasT", [NBROW, 2 * QC], f8, isOutput=False)
    # DoubleRow selector weights: cols 0:256 pick pair slot A, 256:512 slot B
    idpk = nc.declare_dram_parameter("idpk", [P, 4 * P], f8, isOutput=False)
    # W stacked for head pairing: cols 0:768 = [W_h0; W_h1] (128 rows),
    # cols 768:1536 rows 0:64 = W_h2
    wproj = nc.declare_dram_parameter("wproj", [P, 2 * DIM], bf16, isOutput=False)
    out = nc.declare_dram_parameter("out", [T, DIM], f16, isOutput=True)

    with tile.TileContext(nc) as tc, ExitStack() as ctx:
        # bias is streamed in sub-regions of 4 consecutive s-tiles
        # ([128, 4x512] bf16): fine enough that the first compute starts
        # ~4us in and the big j>=1 regions never head-of-line-block the
        # DMA queue.  The first three subs (j=0) go on the SP queue
        # interleaved with the qk/va loads in consumption order; the
        # rest go on the otherwise-idle Pool (SWDGE) queue with a
        # rolling lookahead.
        brow0 = {}  # DRAM row offset of region (j, h)
        off = 0
        for j, h in [(3, 0), (3, 1), (3, 2), (2, 0), (2, 1), (2, 2),
                     (0, 0), (1, 0), (0, 1), (1, 1), (0, 2), (1, 2)]:
            brow0[(j, h)] = off
            off += _bias_rows(j) // 2
        # consumption order: j=3,2 first (thickest compute hides the DMA
        # stream warm-up), then the tiny j=0 regions interleaved between
        # the j=1 regions so their drains don't pile up on the DVE
        region_order = [(3, 0), (3, 1), (3, 2), (2, 0), (2, 1), (2, 2),
                        (0, 0), (1, 0), (0, 1), (1, 1), (0, 2), (1, 2)]
        subs = []  # (j, h, k) in consumption order
        for j, h in region_order:
            for k in range(j + 1):
                subs.append((j, h, k))

        bias_pool = ctx.enter_context(tc.tile_pool(name="bias", bufs=10))
        bias_ts = {}
        nissued = [0]

        def issue_subs(upto, eng):
            while nissued[0] < min(upto, len(subs)):
                j, h, k = subs[nissued[0]]
                b_t = bias_pool.tile([P, 4 * QC], f8, name="bsub_t")
                eng.dma_start(
                    b_t[:].rearrange("p (a q) -> p a q", a=2),
                    biasT[ds(brow0[(j, h)] + k * 2 * P, 2 * P), :].rearrange(
                        "(a p) q -> p a q", p=P
                    ),
                )
                bias_ts[(j, h, k)] = b_t
                nissued[0] += 1

        # startup loads in consumption order, all on the SP queue.
        # qk slab pieces are separate tiles so the first QK only waits
        # on its own piece: A = qT chunk j=3 + kT s-tiles 0:8 for heads
        # 0,1; B2 = kT s-tiles 8:16; B1 = qT chunks j=0,1,2 (consumed
        # much later); head 2 loads as one 64-partition slab.
        qk_pool = ctx.enter_context(tc.tile_pool(name="qk", bufs=5))
        id8_pool = ctx.enter_context(tc.tile_pool(name="id8", bufs=1))
        id8_t = id8_pool.tile([P, 4 * P], f8)
        nc.sync.dma_start(id8_t[:], idpk[:])
        slabA1 = qk_pool.tile([P, QC + 2 * P], bf16, name="slabA1")
        nc.sync.dma_start(slabA1[:], qk[0:P, 3 * QC : 4 * QC + 2 * P])
        issue_subs(1, nc.sync)
        va_pool = ctx.enter_context(tc.tile_pool(name="va", bufs=HPC + 1))
        va_ts = [None]

        def load_va(h):
            va_t = va_pool.tile([P, T], bf16, name="va_t")
            nc.sync.dma_start(va_t[:], va[ds(h * P, P), :])
            va_ts.append(va_t)

        va0a = va_pool.tile([P, 4 * P], bf16, name="va0a")
        nc.sync.dma_start(va0a[:], va[0:P, 0 : 4 * P])
        issue_subs(2, nc.sync)
        va0b = va_pool.tile([P, T - 4 * P], bf16, name="va0b")
        nc.sync.dma_start(va0b[:], va[0:P, 4 * P : T])
        slabA2 = qk_pool.tile([P, 2 * QC - 2 * P], bf16, name="slabA2")
        nc.sync.dma_start(slabA2[:], qk[0:P, 4 * QC + 2 * P : 6 * QC])
        slabB2 = qk_pool.tile([P, 2 * QC], bf16, name="slabB2")
        nc.sync.dma_start(slabB2[:], qk[0:P, 6 * QC : 8 * QC])
        issue_subs(4, nc.sync)
        load_va(1)
        slab2 = qk_pool.tile([D, 2 * T], bf16, name="slab2")
        nc.sync.dma_start(slab2[:], qk[ds(P, D), :])
        load_va(2)
        slabB1 = qk_pool.tile([P, 3 * QC], bf16, name="slabB1")
        nc.sync.dma_start(slabB1[:], qk[0:P, 0 : 3 * QC])

        def va_ap(h, i):
            if h == 0:
                if i < 4:
                    return va0a[:, i * P : (i + 1) * P]
                return va0b[:, (i - 4) * P : (i - 3) * P]
            return va_ts[h][:, i * P : (i + 1) * P]

        def qT_ap(h, j):
            if h == 2:
                return slab2[:, j * QC : (j + 1) * QC]
            r = slice(h * D, (h + 1) * D)
            if j == 3:
                return slabA1[r, 0:QC]
            return slabB1[r, j * QC : (j + 1) * QC]

        def kT_ap(h, i):
            if h == 2:
                return slab2[:, 4 * QC + i * P : 4 * QC + (i + 1) * P]
            r = slice(h * D, (h + 1) * D)
            if i < 2:
                return slabA1[r, QC + i * P : QC + (i + 1) * P]
            if i < 8:
                return slabA2[r, (i - 2) * P : (i - 1) * P]
            return slabB2[r, (i - 8) * P : (i - 7) * P]

        w_pool = ctx.enter_context(tc.tile_pool(name="w", bufs=1))
        w_all = w_pool.tile([P, 2 * DIM], bf16)
        nc.sync.dma_start(w_all[:], wproj[:])
        w01 = w_all[:, 0:DIM]
        w2 = w_all[0:D, DIM : 2 * DIM]

        with (
            tc.tile_pool(name="pexp", bufs=4) as pexp_pool,
            tc.tile_pool(name="rec", bufs=2) as rec_pool,
            tc.tile_pool(name="yt01", bufs=2) as yt01_pool,
            tc.tile_pool(name="yt2", bufs=2) as yt2_pool,
            tc.tile_pool(name="ob", bufs=4) as ob_pool,
            tc.tile_pool(name="psl", bufs=3, space="PSUM") as psl_pool,
            tc.tile_pool(name="psy", bufs=2, space="PSUM") as psy_pool,
            tc.tile_pool(name="pspa", bufs=2, space="PSUM") as pspa_pool,
            tc.tile_pool(name="pspb", bufs=1, space="PSUM") as pspb_pool,
        ):
            yt_box = {}

            def emit_drain(j, h, psy_t):
                # rows 64:128 of psy hold the softmax denominators
                # (replicated); realign to partitions 0:64 via the DVE
                # output crossbar while taking the reciprocal.
                if h == 0:
                    yt_box[j] = (
                        yt01_pool.tile([P, QC], bf16, name="yt01_t"),
                        yt2_pool.tile([D, QC], bf16, name="yt2_t"),
                    )
                yt01_t, yt2_t = yt_box[j]
                rec_t = rec_pool.tile([D, QC], f32)
                nc.vector.reciprocal(rec_t[:], psy_t[D : 2 * D, :])
                if h < 2:
                    dst = yt01_t[h * D : (h + 1) * D, :]
                else:
                    dst = yt2_t[:]
                nc.vector.tensor_mul(dst, psy_t[0:D, :], rec_t[:])

            def mk_proj(j, tb, yt01_t, yt2_t, copy_eng):
                # one projection t-block; queued so the t-blocks interleave
                # with the next region's attention tiles (the ACT engine
                # starves if all 4 run back-to-back on the in-order PE)
                def emit():
                    pa_t = pspa_pool.tile([P, QC], f32, name="pa_t")
                    pb_t = pspb_pool.tile([P, DIM - QC], f32, name="pb_t")
                    ob_t = ob_pool.tile([P, DIM], f16, name="ob_t")
                    for psp, o0, ow in (
                        (pa_t, 0, QC),
                        (pb_t, QC, DIM - QC),
                    ):
                        nc.tensor.matmul(
                            psp[:],
                            lhsT=yt01_t[:, tb * P : (tb + 1) * P],
                            rhs=w01[:, o0 : o0 + ow],
                            start=True,
                            stop=False,
                        )
                        nc.tensor.matmul(
                            psp[:],
                            lhsT=yt2_t[:, tb * P : (tb + 1) * P],
                            rhs=w2[:, o0 : o0 + ow],
                            start=False,
                            stop=True,
                        )
                        copy_eng(ob_t[:, o0 : o0 + ow], psp[:])
                    nc.sync.dma_start(
                        out[ds(j * QC + tb * P, P), :], ob_t[:]
                    )
                return emit

            order = [(3, 0), (3, 1), (3, 2), (2, 0), (2, 1), (2, 2),
                     (0, 0), (1, 0), (0, 1), (1, 1), (0, 2), (1, 2)]
            # software pipeline: the PV of logits-tile i is emitted LAG
            # tiles after its QK/copy/exp, so the in-order PE queue always
            # has matmul work while the ACT engine runs the exps, and the
            # chunk-j projection trails one tile further still
            from collections import deque

            LAG = 2
            queue = deque()
            sub0 = {}  # first sub index of region (j, h)
            for n, (j, h, k) in enumerate(subs):
                if k == 0:
                    sub0[(j, h)] = n
            for j, h in order:
                issue_subs(sub0[(j, h)] + (j + 1) + 4, nc.gpsimd)
                qT_t = qT_ap(h, j)
                psy_box = [None]
                ntile = (j + 1) * 4
                for i in range(ntile):
                    c0 = max(0, P * i - QC * j)
                    psl_t = psl_pool.tile([P, QC], f32)
                    # QK first (can run before the bias arrives)
                    nc.tensor.matmul(
                        psl_t[:, c0:QC],
                        lhsT=kT_ap(h, i),
                        rhs=qT_t[:, c0:QC],
                        start=True,
                        stop=False,
                    )
                    # bias lands on top: fp8 DoubleRow identity copy
                    # (0.5 PE cycles/row); tile pairs are column-interleaved
                    # in SBUF, the selector weights pick one tile per instr
                    pr, sl = (i % 4) // 2, i % 2
                    nc.tensor.matmul(
                        psl_t[:, c0:QC],
                        lhsT=id8_t[
                            :, sl * 2 * P : (sl + 1) * 2 * P
                        ].rearrange("p (two m) -> p two m", two=2),
                        rhs=bias_ts[(j, h, i // 4)][
                            :, pr * 2 * QC : (pr + 1) * 2 * QC
                        ].rearrange("p (two q) -> p two q", two=2)[:, :, c0:],
                        start=False,
                        stop=True,
                        perf_mode=mybir.MatmulPerfMode.DoubleRow,
                    )
                    pe_t = pexp_pool.tile([P, QC], bf16)
                    nc.scalar.activation(
                        pe_t[:, c0:QC], psl_t[:, c0:QC], EXP
                    )

                    def mk_pv(j, h, i, c0, pe_t, psy_box):
                        def emit():
                            if i == 0:
                                psy_box[0] = psy_pool.tile(
                                    [P, QC], f32, name="psy_t"
                                )
                            psy_t = psy_box[0]
                            nc.tensor.matmul(
                                psy_t[:, c0:QC],
                                lhsT=va_ap(h, i),
                                rhs=pe_t[:, c0:QC],
                                start=(i == 0),
                                stop=(i == 4 * j + 3),
                            )
                            if i == 4 * j + 3:
                                emit_drain(j, h, psy_t)
                                if h == 2:
                                    final = (j, h) == order[-1]
                                    for tb in range(4):
                                        ce = (
                                            nc.scalar.copy
                                            if final and tb % 2 == 0
                                            else nc.vector.tensor_copy
                                        )
                                        pj = mk_proj(j, tb, *yt_box[j], ce)
                                        if final:
                                            pj()
                                        else:
                                            queue.append(pj)
                        return emit

                    queue.append(mk_pv(j, h, i, c0, pe_t, psy_box))
                    while len(queue) > LAG:
                        queue.popleft()()
            while queue:
                queue.popleft()()

    nc.finalize()
    return nc


def _get_program():
    global _PROGRAM
    if _PROGRAM is None:
        _PROGRAM = _build_program()
    return _PROGRAM


def make_in_maps(q, k, v, attn_bias, W_proj):
    """Host-side sharding/layout prep: one input map per core."""
    q = np.asarray(q, dtype=np.float32)
    k = np.asarray(k, dtype=np.float32)
    v = np.asarray(v, dtype=np.float32)
    attn_bias = np.asarray(attn_bias, dtype=np.float32)
    W_proj = np.asarray(W_proj, dtype=np.float32)

    scale = 1.0 / math.sqrt(D)
    # causal mask in transposed [s, q] coords: masked where s > q
    smask = (np.arange(T)[:, None] > np.arange(T)[None, :]).astype(np.float32)
    smask *= -10000.0
    w_heads = W_proj.reshape(H, D, DIM)

    in_maps = []
    for c in range(NCORES):
        b = c // 4
        h0 = HPC * (c % 4)
        hs = slice(h0, h0 + HPC)
        qk = np.zeros((P + D, 2 * T), dtype=ml_dtypes.bfloat16)
        for hh in range(HPC):
            qk[hh * D : (hh + 1) * D, :T] = (
                q[b, h0 + hh].transpose(1, 0) * scale
            )
            qk[hh * D : (hh + 1) * D, T:] = k[b, h0 + hh].transpose(1, 0)
        va = np.zeros((HPC, P, NT, P), dtype=ml_dtypes.bfloat16)
        va[:, :, :, :D] = v[b, hs].reshape(HPC, NT, P, D).transpose(0, 2, 1, 3)
        va[:, :, :, D:] = 1.0
        biasT = attn_bias[b, hs].transpose(0, 2, 1) + smask[None]
        biasT = np.clip(biasT, -240.0, 240.0).astype(ml_dtypes.float8_e4m3)
        # region (j, h): s rows [0:(j+1)*512), q cols [j*512:(j+1)*512),
        # s-tile pairs column-interleaved for the DoubleRow copy
        regions = []
        for j, h in [(3, 0), (3, 1), (3, 2), (2, 0), (2, 1), (2, 2),
                     (0, 0), (1, 0), (0, 1), (1, 1), (0, 2), (1, 2)]:
            if True:
                r = biasT[h, 0 : (j + 1) * QC, j * QC : (j + 1) * QC]
                pairs = r.reshape((j + 1) * 2, 2, P, QC)
                regions.append(
                    pairs.transpose(0, 2, 1, 3).reshape((j + 1) * 2 * P, 2 * QC)
                )
        bias_pack = np.ascontiguousarray(np.concatenate(regions, axis=0))
        idp = np.zeros((P, 4 * P), dtype=ml_dtypes.float8_e4m3)
        eye = np.eye(P, dtype=np.float32)
        idp[:, 0:P] = eye        # selector A = [I | 0]
        idp[:, 3 * P :] = eye    # selector B = [0 | I]
        wp = np.zeros((P, 2 * DIM), dtype=ml_dtypes.bfloat16)
        wp[:D, :DIM] = w_heads[h0]
        wp[D:, :DIM] = w_heads[h0 + 1]
        wp[:D, DIM:] = w_heads[h0 + 2]
        in_maps.append(
            {
                "qk": qk,
                "va": va.reshape(HPC * P, T),
                "biasT": bias_pack,
                "idpk": idp,
                "wproj": wp,
            }
        )
    return in_maps


def assemble_output(results):
    """Sum the 4 per-core fp16 partial projections for each batch."""
    out = np.zeros((B, T, DIM), dtype=np.float32)
    for c in range(NCORES):
        out[c // 4] += results[c]["out"].astype(np.float32)
    return out


def kernel(q, k, v, attn_bias, W_proj):
    from concourse.bass_utils import run_bass_kernel_spmd

    nc = _get_program()
    in_maps = make_in_maps(q, k, v, attn_bias, W_proj)
    res = run_bass_kernel_spmd(nc, in_maps, list(range(NCORES)))
    return assemble_output(res.results)


# revision 32
# speedup vs baseline: 1.8835x; 1.0072x over previous
"""Causal attention + output projection on 8 Trainium2 NeuronCores.

Problem (hardcoded): B=2, H=12, T=2048, D=64, DIM=768, fp32.

Sharding: 24 (b, h) pairs -> 3 heads per core; cores 0-3 take b=0,
cores 4-7 take b=1.  Each core computes attention for its 3 heads plus
the partial output projection  sum_h y_h @ W[h*64:(h+1)*64, :]  as a
(T, DIM) fp16 partial; the host sums the 4 partials per batch.  No
collectives.

Device-side layout is fully transposed ([s, q]) so no on-chip
transposes are needed:
  - host feeds qk = [qT/sqrt(D); kT] stacked on 128 partitions (bf16)
  - host feeds biasT = bias^T with the causal mask pre-added
    (-1e4 on s > q) in bf16, packed per (q-chunk j, head) region
  - v is fed augmented with 64 ones-columns (bf16) so a single PV
    matmul yields both y^T (rows 0:64) and the softmax denominators
    replicated across rows 64:128.

All matmuls are bf16 (1 PE cycle/row vs 4 for fp32); PSUM accumulation
stays fp32.  The loop is q-chunk-major (j outer, head inner) so the
output projection and its DMA for chunk j overlap the attention of
chunk j+1.  Per (j, head, group of 2 s-tiles):
  PSUM[s=128, q<=512] <- kT-tile.T @ qT-chunk  (causally trimmed)
  PSUM               += identity @ biasT-tile  (bias copy, trimmed)
  SBUF P = exp(PSUM)   (bf16 out; one ACT instruction per PSUM bank)
  PSUM_y[128, 512]   += vaug-tile.T @ P-slice  (accumulated over s)
then  rec = 1/sums  (DVE partition-realigning read 64:128 -> 0:64),
      yTj[h rows] = y_un * rec  (heads 0,1 stacked on 128 partitions
so the projection contracts 128 rows per matmul).
Projection per t-block: psp = yTj.T @ [W_h0; W_h1] + yTj2.T @ W_h2,
copied to fp16 and DMA'd per chunk.

Bias DMAs are issued on the otherwise-idle Pool (SWDGE) queue, one DMA
per (j, head) in consumption order; qk/va/W loads and output stores go
on the SP queue.
"""

import math

import numpy as np
import ml_dtypes

B, H, T, D = 2, 12, 2048, 64
DIM = H * D
NCORES = 8
HPC = 3           # heads per core
P = 128
QC = 512          # q-chunk width (one PSUM bank of fp32)
NJ = T // QC      # 4 q-chunks
NT = T // P       # 16 s-tiles
GROUP = 2         # s-tiles per PSUM logits group (2 banks)

_PROGRAM = None


def _bias_rows(j):
    """Number of 128-row s-tile rows in the (j, h) bias region."""
    return (j + 1) * 4 * P


def _build_program():
    import concourse.bass as bass
    import concourse.mybir as mybir
    import concourse.tile as tile
    from concourse import bacc
    from contextlib import ExitStack

    dt = mybir.dt
    f32 = dt.float32
    f16 = dt.float16
    bf16 = dt.bfloat16
    f8 = dt.float8e4
    EXP = mybir.ActivationFunctionType.Exp
    ds = bass.ds

    nc = bacc.Bacc("TRN2", num_devices=NCORES)
    # [qT | kT] slabs, bf16: rows 0:128 = heads 0,1 stacked on 128
    # partitions (full-rate DMA), rows 128:192 = head 2
    qk = nc.declare_dram_parameter("qk", [P + D, 2 * T], bf16, isOutput=False)
    va = nc.declare_dram_parameter("va", [HPC * P, T], bf16, isOutput=False)
    # bias regions packed j-major then head: region (j,h) is the
    # [s=0:(j+1)*512, q=j*512:(j+1)*512] block of head h's biasT,
    # flattened s-tile-major
    NBROW = sum(_bias_rows(j) for j in range(NJ)) * HPC // 2
    biasT = nc.declare_dram_parameter("biasT", [NBROW, 2 * QC], f8, isOutput=False)
    # DoubleRow selector weights: cols 0:256 pick pair slot A, 256:512 slot B
    idpk = nc.declare_dram_parameter("idpk", [P, 4 * P], f8, isOutput=False)
    # W stacked for head pairing: cols 0:768 = [W_h0; W_h1] (128 rows),
    # cols 768:1536 rows 0:64 = W_h2
    wproj = nc.declare_dram_parameter("wproj", [P, 2 * DIM], bf16, isOutput=False)
    out = nc.declare_dram_parameter("out", [T, DIM], f16, isOutput=True)

    with tile.TileContext(nc) as tc, ExitStack() as ctx:
        # bias is streamed in sub-regions of 4 consecutive s-tiles
        # ([128, 4x512] bf16): fine enough that the first compute starts
        # ~4us in and the big j>=1 regions never head-of-line-block the
        # DMA queue.  The first three subs (j=0) go on the SP queue
        # interleaved with the qk/va loads in consumption order; the
        # rest go on the otherwise-idle Pool (SWDGE) queue with a
        # rolling lookahead.
        brow0 = {}  # DRAM row offset of region (j, h)
        off = 0
        for j, h in [(3, 0), (3, 1), (3, 2), (0, 0), (2, 0), (0, 1),
                     (2, 1), (0, 2), (2, 2), (1, 0), (1, 1), (1, 2)]:
            brow0[(j, h)] = off
            off += _bias_rows(j) // 2
        # consumption order: j=3,2 first (thickest compute hides the DMA
        # stream warm-up), then the tiny j=0 regions interleaved between
        # the j=1 regions so their drains don't pile up on the DVE
        region_order = [(3, 0), (3, 1), (3, 2), (0, 0), (2, 0), (0, 1),
                        (2, 1), (0, 2), (2, 2), (1, 0), (1, 1), (1, 2)]
        subs = []  # (j, h, k) in consumption order
        for j, h in region_order:
            for k in range(j + 1):
                subs.append((j, h, k))

        bias_pool = ctx.enter_context(tc.tile_pool(name="bias", bufs=30))
        bias_ts = {}
        nissued = [0]

        def issue_subs(upto, eng):
            while nissued[0] < min(upto, len(subs)):
                j, h, k = subs[nissued[0]]
                b_t = bias_pool.tile([P, 4 * QC], f8, name="bsub_t")
                eng.dma_start(
                    b_t[:].rearrange("p (a q) -> p a q", a=2),
                    biasT[ds(brow0[(j, h)] + k * 2 * P, 2 * P), :].rearrange(
                        "(a p) q -> p a q", p=P
                    ),
                )
                bias_ts[(j, h, k)] = b_t
                nissued[0] += 1

        # startup loads in consumption order, all on the SP queue.
        # qk slab pieces are separate tiles so the first QK only waits
        # on its own piece: A = qT chunk j=3 + kT s-tiles 0:8 for heads
        # 0,1; B2 = kT s-tiles 8:16; B1 = qT chunks j=0,1,2 (consumed
        # much later); head 2 loads as one 64-partition slab.
        qk_pool = ctx.enter_context(tc.tile_pool(name="qk", bufs=5))
        id8_pool = ctx.enter_context(tc.tile_pool(name="id8", bufs=1))
        id8_t = id8_pool.tile([P, 4 * P], f8)
        nc.sync.dma_start(id8_t[:], idpk[:])
        slabA1 = qk_pool.tile([P, QC + 2 * P], bf16, name="slabA1")
        nc.sync.dma_start(slabA1[:], qk[0:P, 3 * QC : 4 * QC + 2 * P])
        issue_subs(1, nc.sync)
        va_pool = ctx.enter_context(tc.tile_pool(name="va", bufs=HPC + 1))
        va_ts = [None]

        def load_va(h):
            va_t = va_pool.tile([P, T], bf16, name="va_t")
            nc.sync.dma_start(va_t[:], va[ds(h * P, P), :])
            va_ts.append(va_t)

        va0a = va_pool.tile([P, 4 * P], bf16, name="va0a")
        nc.sync.dma_start(va0a[:], va[0:P, 0 : 4 * P])
        issue_subs(2, nc.sync)
        va0b = va_pool.tile([P, T - 4 * P], bf16, name="va0b")
        nc.sync.dma_start(va0b[:], va[0:P, 4 * P : T])
        slabA2 = qk_pool.tile([P, 2 * QC - 2 * P], bf16, name="slabA2")
        nc.sync.dma_start(slabA2[:], qk[0:P, 4 * QC + 2 * P : 6 * QC])
        slabB2 = qk_pool.tile([P, 2 * QC], bf16, name="slabB2")
        nc.sync.dma_start(slabB2[:], qk[0:P, 6 * QC : 8 * QC])
        issue_subs(4, nc.sync)
        load_va(1)
        slab2 = qk_pool.tile([D, 2 * T], bf16, name="slab2")
        nc.sync.dma_start(slab2[:], qk[ds(P, D), :])
        load_va(2)
        slabB1 = qk_pool.tile([P, 3 * QC], bf16, name="slabB1")
        nc.sync.dma_start(slabB1[:], qk[0:P, 0 : 3 * QC])

        def va_ap(h, i):
            if h == 0:
                if i < 4:
                    return va0a[:, i * P : (i + 1) * P]
                return va0b[:, (i - 4) * P : (i - 3) * P]
            return va_ts[h][:, i * P : (i + 1) * P]

        def qT_ap(h, j):
            if h == 2:
                return slab2[:, j * QC : (j + 1) * QC]
            r = slice(h * D, (h + 1) * D)
            if j == 3:
                return slabA1[r, 0:QC]
            return slabB1[r, j * QC : (j + 1) * QC]

        def kT_ap(h, i):
            if h == 2:
                return slab2[:, 4 * QC + i * P : 4 * QC + (i + 1) * P]
            r = slice(h * D, (h + 1) * D)
            if i < 2:
                return slabA1[r, QC + i * P : QC + (i + 1) * P]
            if i < 8:
                return slabA2[r, (i - 2) * P : (i - 1) * P]
            return slabB2[r, (i - 8) * P : (i - 7) * P]

        w_pool = ctx.enter_context(tc.tile_pool(name="w", bufs=1))
        w_all = w_pool.tile([P, 2 * DIM], bf16)
        nc.sync.dma_start(w_all[:], wproj[:])
        w01 = w_all[:, 0:DIM]
        w2 = w_all[0:D, DIM : 2 * DIM]

        with (
            tc.tile_pool(name="pexp", bufs=4) as pexp_pool,
            tc.tile_pool(name="rec", bufs=2) as rec_pool,
            tc.tile_pool(name="yt01", bufs=2) as yt01_pool,
            tc.tile_pool(name="yt2", bufs=2) as yt2_pool,
            tc.tile_pool(name="ob", bufs=4) as ob_pool,
            tc.tile_pool(name="psl", bufs=3, space="PSUM") as psl_pool,
            tc.tile_pool(name="psy", bufs=2, space="PSUM") as psy_pool,
            tc.tile_pool(name="pspa", bufs=2, space="PSUM") as pspa_pool,
            tc.tile_pool(name="pspb", bufs=1, space="PSUM") as pspb_pool,
        ):
            yt_box = {}

            def emit_drain(j, h, psy_t):
                # rows 64:128 of psy hold the softmax denominators
                # (replicated); realign to partitions 0:64 via the DVE
                # output crossbar while taking the reciprocal.
                if h == 0:
                    yt_box[j] = (
                        yt01_pool.tile([P, QC], bf16, name="yt01_t"),
                        yt2_pool.tile([D, QC], bf16, name="yt2_t"),
                    )
                yt01_t, yt2_t = yt_box[j]
                rec_t = rec_pool.tile([D, QC], f32)
                nc.vector.reciprocal(rec_t[:], psy_t[D : 2 * D, :])
                if h < 2:
                    dst = yt01_t[h * D : (h + 1) * D, :]
                else:
                    dst = yt2_t[:]
                nc.vector.tensor_mul(dst, psy_t[0:D, :], rec_t[:])

            def mk_proj(j, tb, yt01_t, yt2_t, copy_eng):
                # one projection t-block; queued so the t-blocks interleave
                # with the next region's attention tiles (the ACT engine
                # starves if all 4 run back-to-back on the in-order PE)
                def emit():
                    pa_t = pspa_pool.tile([P, QC], f32, name="pa_t")
                    pb_t = pspb_pool.tile([P, DIM - QC], f32, name="pb_t")
                    ob_t = ob_pool.tile([P, DIM], f16, name="ob_t")
                    for psp, o0, ow in (
                        (pa_t, 0, QC),
                        (pb_t, QC, DIM - QC),
                    ):
                        nc.tensor.matmul(
                            psp[:],
                            lhsT=yt01_t[:, tb * P : (tb + 1) * P],
                            rhs=w01[:, o0 : o0 + ow],
                            start=True,
                            stop=False,
                        )
                        nc.tensor.matmul(
                            psp[:],
                            lhsT=yt2_t[:, tb * P : (tb + 1) * P],
                            rhs=w2[:, o0 : o0 + ow],
                            start=False,
                            stop=True,
                        )
                        copy_eng(ob_t[:, o0 : o0 + ow], psp[:])
                    nc.sync.dma_start(
                        out[ds(j * QC + tb * P, P), :], ob_t[:]
                    )
                return emit

            order = [(3, 0), (3, 1), (3, 2), (0, 0), (2, 0), (0, 1),
                     (2, 1), (0, 2), (2, 2), (1, 0), (1, 1), (1, 2)]
            # software pipeline: the PV of logits-tile i is emitted LAG
            # tiles after its QK/copy/exp, so the in-order PE queue always
            # has matmul work while the ACT engine runs the exps, and the
            # chunk-j projection trails one tile further still
            from collections import deque

            LAG = 2
            queue = deque()
            sub0 = {}  # first sub index of region (j, h)
            for n, (j, h, k) in enumerate(subs):
                if k == 0:
                    sub0[(j, h)] = n
            for j, h in order:
                issue_subs(sub0[(j, h)] + (j + 1) + 4, nc.gpsimd)
                qT_t = qT_ap(h, j)
                psy_box = [None]
                ntile = (j + 1) * 4
                for i in range(ntile):
                    c0 = max(0, P * i - QC * j)
                    psl_t = psl_pool.tile([P, QC], f32)
                    # QK first (can run before the bias arrives)
                    nc.tensor.matmul(
                        psl_t[:, c0:QC],
                        lhsT=kT_ap(h, i),
                        rhs=qT_t[:, c0:QC],
                        start=True,
                        stop=False,
                    )
                    # bias on top: fp8 DoubleRow identity copy (0.5 PE
                    # cycles/row); tile pairs are column-blocked in SBUF,
                    # the selector weights pick one tile per instruction
                    pr, sl = (i % 4) // 2, i % 2
                    nc.tensor.matmul(
                        psl_t[:, c0:QC],
                        lhsT=id8_t[
                            :, sl * 2 * P : (sl + 1) * 2 * P
                        ].rearrange("p (two m) -> p two m", two=2),
                        rhs=bias_ts[(j, h, i // 4)][
                            :, pr * 2 * QC : (pr + 1) * 2 * QC
                        ].rearrange("p (two q) -> p two q", two=2)[:, :, c0:],
                        start=False,
                        stop=True,
                        perf_mode=mybir.MatmulPerfMode.DoubleRow,
                    )
                    pe_t = pexp_pool.tile([P, QC], bf16)
                    nc.scalar.activation(
                        pe_t[:, c0:QC], psl_t[:, c0:QC], EXP
                    )

                    def mk_pv(j, h, i, c0, pe_t, psy_box):
                        def emit():
                            if i == 0:
                                psy_box[0] = psy_pool.tile(
                                    [P, QC], f32, name="psy_t"
                                )
                            psy_t = psy_box[0]
                            nc.tensor.matmul(
                                psy_t[:, c0:QC],
                                lhsT=va_ap(h, i),
                                rhs=pe_t[:, c0:QC],
                                start=(i == 0),
                                stop=(i == 4 * j + 3),
                            )
                            if i == 4 * j + 3:
                                emit_drain(j, h, psy_t)
                                if h == 2:
                                    final = (j, h) == order[-1]
                                    for tb in range(4):
                                        ce = (
                                            nc.scalar.copy
                                            if final and tb % 2 == 0
                                            else nc.vector.tensor_copy
                                        )
                                        pj = mk_proj(j, tb, *yt_box[j], ce)
                                        if final:
                                            pj()
                                        else:
                                            queue.append(pj)
                        return emit

                    queue.append(mk_pv(j, h, i, c0, pe_t, psy_box))
                    while len(queue) > LAG:
                        queue.popleft()()
            while queue:
                queue.popleft()()

    nc.finalize()
    return nc


def _get_program():
    global _PROGRAM
    if _PROGRAM is None:
        _PROGRAM = _build_program()
    return _PROGRAM


def make_in_maps(q, k, v, attn_bias, W_proj):
    """Host-side sharding/layout prep: one input map per core."""
    q = np.asarray(q, dtype=np.float32)
    k = np.asarray(k, dtype=np.float32)
    v = np.asarray(v, dtype=np.float32)
    attn_bias = np.asarray(attn_bias, dtype=np.float32)
    W_proj = np.asarray(W_proj, dtype=np.float32)

    scale = 1.0 / math.sqrt(D)
    # causal mask in transposed [s, q] coords: masked where s > q
    smask = (np.arange(T)[:, None] > np.arange(T)[None, :]).astype(np.float32)
    smask *= -10000.0
    w_heads = W_proj.reshape(H, D, DIM)

    in_maps = []
    for c in range(NCORES):
        b = c // 4
        h0 = HPC * (c % 4)
        hs = slice(h0, h0 + HPC)
        qk = np.zeros((P + D, 2 * T), dtype=ml_dtypes.bfloat16)
        for hh in range(HPC):
            qk[hh * D : (hh + 1) * D, :T] = (
                q[b, h0 + hh].transpose(1, 0) * scale
            )
            qk[hh * D : (hh + 1) * D, T:] = k[b, h0 + hh].transpose(1, 0)
        va = np.zeros((HPC, P, NT, P), dtype=ml_dtypes.bfloat16)
        va[:, :, :, :D] = v[b, hs].reshape(HPC, NT, P, D).transpose(0, 2, 1, 3)
        va[:, :, :, D:] = 1.0
        biasT = attn_bias[b, hs].transpose(0, 2, 1) + smask[None]
        biasT = np.clip(biasT, -240.0, 240.0).astype(ml_dtypes.float8_e4m3)
        # region (j, h): s rows [0:(j+1)*512), q cols [j*512:(j+1)*512),
        # s-tile pairs column-interleaved for the DoubleRow copy
        regions = []
        for j, h in [(3, 0), (3, 1), (3, 2), (0, 0), (2, 0), (0, 1),
                     (2, 1), (0, 2), (2, 2), (1, 0), (1, 1), (1, 2)]:
            if True:
                r = biasT[h, 0 : (j + 1) * QC, j * QC : (j + 1) * QC]
                pairs = r.reshape((j + 1) * 2, 2, P, QC)
                regions.append(
                    pairs.transpose(0, 2, 1, 3).reshape((j + 1) * 2 * P, 2 * QC)
                )
        bias_pack = np.ascontiguousarray(np.concatenate(regions, axis=0))
        idp = np.zeros((P, 4 * P), dtype=ml_dtypes.float8_e4m3)
        eye = np.eye(P, dtype=np.float32)
        idp[:, 0:P] = eye        # selector A = [I | 0]
        idp[:, 3 * P :] = eye    # selector B = [0 | I]
        wp = np.zeros((P, 2 * DIM), dtype=ml_dtypes.bfloat16)
        wp[:D, :DIM] = w_heads[h0]
        wp[D:, :DIM] = w_heads[h0 + 1]
        wp[:D, DIM:] = w_heads[h0 + 2]
        in_maps.append(
            {
                "qk": qk,
                "va": va.reshape(HPC * P, T),
                "biasT": bias_pack,
                "idpk": idp,
                "wproj": wp,
            }
        )
    return in_maps


def assemble_output(results):
    """Sum the 4 per-core fp16 partial projections for each batch."""
    out = np.zeros((B, T, DIM), dtype=np.float32)
    for c in range(NCORES):
        out[c // 4] += results[c]["out"].astype(np.float32)
    return out


def kernel(q, k, v, attn_bias, W_proj):
    from concourse.bass_utils import run_bass_kernel_spmd

    nc = _get_program()
    in_maps = make_in_maps(q, k, v, attn_bias, W_proj)
    res = run_bass_kernel_spmd(nc, in_maps, list(range(NCORES)))
    return assemble_output(res.results)


# revision 46
# speedup vs baseline: 1.9181x; 1.0184x over previous
"""Causal attention + output projection on 8 Trainium2 NeuronCores.

Problem (hardcoded): B=2, H=12, T=2048, D=64, DIM=768, fp32.
Modeled single-core time 81.0us (baseline fp32 kernel: 369.2us).

Sharding: 24 (b, h) pairs -> 3 heads per core; cores 0-3 take b=0,
cores 4-7 take b=1.  Each core computes attention for its 3 heads plus
the partial output projection  sum_h y_h @ W[h*64:(h+1)*64, :]  as a
(T, DIM) fp16 partial; the host sums the 4 partials per batch.  No
collectives.

Device-side layout is fully transposed ([s, q]) so no on-chip
transposes are needed:
  - host feeds [qT/sqrt(D) | kT] slabs in bf16: heads 0,1 stacked on
    128 partitions (full-rate DMA), head 2 on partitions 0:64
  - host feeds biasT = bias^T with the causal mask pre-added (clamped
    to -240) in fp8 e4m3, packed per (q-chunk, head) region with
    s-tile pairs column-blocked for the DoubleRow copy
  - v is fed augmented with 64 ones-columns (bf16) so a single PV
    matmul yields both y^T (rows 0:64) and the softmax denominators
    replicated across rows 64:128.

Engine budget per core: PE 64.6us, ACT (exp) 67.4us, DVE ~36us, DMA
~16.5 MB.  All matmuls are bf16 (1 PE cycle/row vs 4 for fp32) except
the bias load into PSUM, which is an fp8 DoubleRow identity-matmul
copy (0.5 cycles/row): the packed pair supplies two s-tiles per
partition-row and a [I|0] / [0|I] selector picks one per instruction.
PSUM accumulation stays fp32.  Per logits tile ([s=128, q<=512], all
stages trimmed to the causal region):
  PSUM  <- kT-tile.T @ qT-chunk    (QK first; runs before bias lands)
  PSUM  += DoubleRow identity @ biasT-pair   (fp8)
  SBUF P = exp(PSUM)               (one ACT instruction, bf16 out)
  PSUM_y += vaug-tile.T @ P-slice  (accumulated over s)
then rec = 1/sums in two halves (DVE partition-realigning read
64:128 -> 0:64), yT = y_un * rec with heads 0,1 stacked on 128
partitions so the projection contracts 128 rows per matmul.

Scheduling: a software pipeline emits the PV of tile i LAG=4 tiles
after its QK/copy/exp so the in-order PE queue always has matmul work
while ACT runs the exps; the per-t-block projection (psp = yT01.T @
[W_h0;W_h1] + yT2.T @ W_h2, fp16 out, one store DMA per t-block) is
pushed through the same queue so it interleaves with the next chunk's
attention instead of starving ACT.  Region order (3,*),(0/2
interleaved),(1,*) puts the thickest compute first (hides the DMA
stream warm-up) and spaces the tiny j=0 drains.  The epilogue emits
the final chunk's yt01-side projection matmuls before the last drain
and splits that drain into halves to shorten the tail.

Bias streams in 4-s-tile sub-region DMAs: the j=0 pieces on the SP
queue interleaved with qk/va pieces in consumption order, the rest on
the otherwise-idle Pool (SWDGE) queue; every sub-region has its own
SBUF slot so no DMA issue ever blocks on tile reuse.  PSUM: 3 logits
banks + 2 PV accumulators + 3 projection banks.
"""

import math

import numpy as np
import ml_dtypes

B, H, T, D = 2, 12, 2048, 64
DIM = H * D
NCORES = 8
HPC = 3           # heads per core
P = 128
QC = 512          # q-chunk width (one PSUM bank of fp32)
NJ = T // QC      # 4 q-chunks
NT = T // P       # 16 s-tiles

_PROGRAM = None


def _bias_rows(j):
    """Number of 128-row s-tile rows in the (j, h) bias region."""
    return (j + 1) * 4 * P


def _build_program():
    import concourse.bass as bass
    import concourse.mybir as mybir
    import concourse.tile as tile
    from concourse import bacc
    from contextlib import ExitStack

    dt = mybir.dt
    f32 = dt.float32
    f16 = dt.float16
    bf16 = dt.bfloat16
    f8 = dt.float8e4
    EXP = mybir.ActivationFunctionType.Exp
    ds = bass.ds

    nc = bacc.Bacc("TRN2", num_devices=NCORES)
    # [qT | kT] slabs, bf16: rows 0:128 = heads 0,1 stacked on 128
    # partitions (full-rate DMA), rows 128:192 = head 2
    qk = nc.declare_dram_parameter("qk", [P + D, 2 * T], bf16, isOutput=False)
    va = nc.declare_dram_parameter("va", [HPC * P, T], bf16, isOutput=False)
    # bias regions packed j-major then head: region (j,h) is the
    # [s=0:(j+1)*512, q=j*512:(j+1)*512] block of head h's biasT,
    # flattened s-tile-major
    NBROW = sum(_bias_rows(j) for j in range(NJ)) * HPC // 2
    biasT = nc.declare_dram_parameter("biasT", [NBROW, 2 * QC], f8, isOutput=False)
    # DoubleRow selector weights: cols 0:256 pick pair slot A, 256:512 slot B
    idpk = nc.declare_dram_parameter("idpk", [P, 4 * P], f8, isOutput=False)
    # W stacked for head pairing: cols 0:768 = [W_h0; W_h1] (128 rows),
    # cols 768:1536 rows 0:64 = W_h2
    wproj = nc.declare_dram_parameter("wproj", [P, 2 * DIM], bf16, isOutput=False)
    out = nc.declare_dram_parameter("out", [T, DIM], f16, isOutput=True)

    with tile.TileContext(nc) as tc, ExitStack() as ctx:
        # bias is streamed in sub-regions of 4 consecutive s-tiles
        # ([128, 4x512] bf16): fine enough that the first compute starts
        # ~4us in and the big j>=1 regions never head-of-line-block the
        # DMA queue.  The first three subs (j=0) go on the SP queue
        # interleaved with the qk/va loads in consumption order; the
        # rest go on the otherwise-idle Pool (SWDGE) queue with a
        # rolling lookahead.
        brow0 = {}  # DRAM row offset of region (j, h)
        off = 0
        for j, h in [(3, 0), (3, 1), (3, 2), (0, 0), (2, 0), (0, 1),
                     (2, 1), (0, 2), (2, 2), (1, 0), (1, 1), (1, 2)]:
            brow0[(j, h)] = off
            off += _bias_rows(j) // 2
        # consumption order: j=3,2 first (thickest compute hides the DMA
        # stream warm-up), then the tiny j=0 regions interleaved between
        # the j=1 regions so their drains don't pile up on the DVE
        region_order = [(3, 0), (3, 1), (3, 2), (0, 0), (2, 0), (0, 1),
                        (2, 1), (0, 2), (2, 2), (1, 0), (1, 1), (1, 2)]
        subs = []  # (j, h, k) in consumption order
        for j, h in region_order:
            for k in range(j + 1):
                subs.append((j, h, k))

        bias_pool = ctx.enter_context(tc.tile_pool(name="bias", bufs=30))
        bias_ts = {}
        nissued = [0]

        def issue_subs(upto, eng):
            while nissued[0] < min(upto, len(subs)):
                j, h, k = subs[nissued[0]]
                b_t = bias_pool.tile([P, 4 * QC], f8, name="bsub_t")
                eng.dma_start(
                    b_t[:].rearrange("p (a q) -> p a q", a=2),
                    biasT[ds(brow0[(j, h)] + k * 2 * P, 2 * P), :].rearrange(
                        "(a p) q -> p a q", p=P
                    ),
                )
                bias_ts[(j, h, k)] = b_t
                nissued[0] += 1

        # startup loads in consumption order, all on the SP queue.
        # qk slab pieces are separate tiles so the first QK only waits
        # on its own piece: A = qT chunk j=3 + kT s-tiles 0:8 for heads
        # 0,1; B2 = kT s-tiles 8:16; B1 = qT chunks j=0,1,2 (consumed
        # much later); head 2 loads as one 64-partition slab.
        qk_pool = ctx.enter_context(tc.tile_pool(name="qk", bufs=5))
        id8_pool = ctx.enter_context(tc.tile_pool(name="id8", bufs=1))
        id8_t = id8_pool.tile([P, 4 * P], f8)
        nc.sync.dma_start(id8_t[:], idpk[:])
        slabA1 = qk_pool.tile([P, QC + 2 * P], bf16, name="slabA1")
        nc.sync.dma_start(slabA1[:], qk[0:P, 3 * QC : 4 * QC + 2 * P])
        issue_subs(1, nc.sync)
        va_pool = ctx.enter_context(tc.tile_pool(name="va", bufs=HPC + 1))
        va_ts = [None]

        def load_va(h):
            va_t = va_pool.tile([P, T], bf16, name="va_t")
            nc.sync.dma_start(va_t[:], va[ds(h * P, P), :])
            va_ts.append(va_t)

        va0a = va_pool.tile([P, 4 * P], bf16, name="va0a")
        nc.sync.dma_start(va0a[:], va[0:P, 0 : 4 * P])
        issue_subs(2, nc.sync)
        va0b = va_pool.tile([P, T - 4 * P], bf16, name="va0b")
        nc.sync.dma_start(va0b[:], va[0:P, 4 * P : T])
        slabA2 = qk_pool.tile([P, 2 * QC - 2 * P], bf16, name="slabA2")
        nc.sync.dma_start(slabA2[:], qk[0:P, 4 * QC + 2 * P : 6 * QC])
        slabB2 = qk_pool.tile([P, 2 * QC], bf16, name="slabB2")
        nc.sync.dma_start(slabB2[:], qk[0:P, 6 * QC : 8 * QC])
        issue_subs(4, nc.sync)
        load_va(1)
        slab2 = qk_pool.tile([D, 2 * T], bf16, name="slab2")
        nc.sync.dma_start(slab2[:], qk[ds(P, D), :])
        load_va(2)
        slabB1 = qk_pool.tile([P, 3 * QC], bf16, name="slabB1")
        nc.sync.dma_start(slabB1[:], qk[0:P, 0 : 3 * QC])

        def va_ap(h, i):
            if h == 0:
                if i < 4:
                    return va0a[:, i * P : (i + 1) * P]
                return va0b[:, (i - 4) * P : (i - 3) * P]
            return va_ts[h][:, i * P : (i + 1) * P]

        def qT_ap(h, j):
            if h == 2:
                return slab2[:, j * QC : (j + 1) * QC]
            r = slice(h * D, (h + 1) * D)
            if j == 3:
                return slabA1[r, 0:QC]
            return slabB1[r, j * QC : (j + 1) * QC]

        def kT_ap(h, i):
            if h == 2:
                return slab2[:, 4 * QC + i * P : 4 * QC + (i + 1) * P]
            r = slice(h * D, (h + 1) * D)
            if i < 2:
                return slabA1[r, QC + i * P : QC + (i + 1) * P]
            if i < 8:
                return slabA2[r, (i - 2) * P : (i - 1) * P]
            return slabB2[r, (i - 8) * P : (i - 7) * P]

        w_pool = ctx.enter_context(tc.tile_pool(name="w", bufs=1))
        w_all = w_pool.tile([P, 2 * DIM], bf16)
        nc.sync.dma_start(w_all[:], wproj[:])
        w01 = w_all[:, 0:DIM]
        w2 = w_all[0:D, DIM : 2 * DIM]

        with (
            tc.tile_pool(name="pexp", bufs=6) as pexp_pool,
            tc.tile_pool(name="rec", bufs=2) as rec_pool,
            tc.tile_pool(name="yt01", bufs=2) as yt01_pool,
            tc.tile_pool(name="yt2", bufs=2) as yt2_pool,
            tc.tile_pool(name="ob", bufs=4) as ob_pool,
            tc.tile_pool(name="psl", bufs=3, space="PSUM") as psl_pool,
            tc.tile_pool(name="psy", bufs=2, space="PSUM") as psy_pool,
            tc.tile_pool(name="pspa", bufs=2, space="PSUM") as pspa_pool,
            tc.tile_pool(name="pspb", bufs=1, space="PSUM") as pspb_pool,
        ):
            yt_box = {}

            def emit_drain(j, h, psy_t):
                # rows 64:128 of psy hold the softmax denominators
                # (replicated); realign to partitions 0:64 via the DVE
                # output crossbar while taking the reciprocal.
                if h == 0:
                    yt_box[j] = (
                        yt01_pool.tile([P, QC], bf16, name="yt01_t"),
                        yt2_pool.tile([D, QC], bf16, name="yt2_t"),
                    )
                yt01_t, yt2_t = yt_box[j]
                if h < 2:
                    dst = yt01_t[h * D : (h + 1) * D, :]
                else:
                    dst = yt2_t[:]
                # halves: the psy slot and yt columns release sooner
                for hf in (0, 1):
                    sl = slice(hf * QC // 2, (hf + 1) * QC // 2)
                    rec_t = rec_pool.tile([D, QC // 2], f32, name="rec_t")
                    nc.vector.reciprocal(rec_t[:], psy_t[D : 2 * D, sl])
                    nc.vector.tensor_mul(
                        dst[:, sl], psy_t[0:D, sl], rec_t[:]
                    )

            def mk_proj(j, tb, yt01_t, yt2_t, copy_eng):
                # one projection t-block; queued so the t-blocks interleave
                # with the next region's attention tiles (the ACT engine
                # starves if all 4 run back-to-back on the in-order PE)
                def emit():
                    pa_t = pspa_pool.tile([P, QC], f32, name="pa_t")
                    pb_t = pspb_pool.tile([P, DIM - QC], f32, name="pb_t")
                    ob_t = ob_pool.tile([P, DIM], f16, name="ob_t")
                    for psp, o0, ow in (
                        (pa_t, 0, QC),
                        (pb_t, QC, DIM - QC),
                    ):
                        nc.tensor.matmul(
                            psp[:],
                            lhsT=yt01_t[:, tb * P : (tb + 1) * P],
                            rhs=w01[:, o0 : o0 + ow],
                            start=True,
                            stop=False,
                        )
                        nc.tensor.matmul(
                            psp[:],
                            lhsT=yt2_t[:, tb * P : (tb + 1) * P],
                            rhs=w2[:, o0 : o0 + ow],
                            start=False,
                            stop=True,
                        )
                        copy_eng(ob_t[:, o0 : o0 + ow], psp[:])
                    nc.sync.dma_start(
                        out[ds(j * QC + tb * P, P), :], ob_t[:]
                    )
                return emit

            def emit_final(j, psy_t):
                # epilogue: the yt01-side projection matmuls depend only
                # on heads 0,1 (already drained), so they are emitted
                # BEFORE this head's drain to cover its DVE latency
                yt01_t, _ = yt_box[j]
                pas = {}
                pbs = {}
                for tb in (0, 1):
                    pas[tb] = pspa_pool.tile([P, QC], f32, name="pa_t")
                    nc.tensor.matmul(
                        pas[tb][:],
                        lhsT=yt01_t[:, tb * P : (tb + 1) * P],
                        rhs=w01[:, 0:QC],
                        start=True,
                        stop=False,
                    )
                pbs[0] = pspb_pool.tile([P, DIM - QC], f32, name="pb_t")
                nc.tensor.matmul(
                    pbs[0][:],
                    lhsT=yt01_t[:, 0:P],
                    rhs=w01[:, QC:DIM],
                    start=True,
                    stop=False,
                )
                # split drain: halves release the projection sooner
                _, yt2_t = yt_box[j]
                for hf in (0, 1):
                    sl = slice(hf * QC // 2, (hf + 1) * QC // 2)
                    rec_t = rec_pool.tile([D, QC // 2], f32, name="rec_t")
                    nc.vector.reciprocal(rec_t[:], psy_t[D : 2 * D, sl])
                    nc.vector.tensor_mul(
                        yt2_t[:, sl], psy_t[0:D, sl], rec_t[:]
                    )
                yt01_t, yt2_t = yt_box[j]
                for tb in range(4):
                    pa = pas.get(tb)
                    if pa is None:
                        pa = pspa_pool.tile([P, QC], f32, name="pa_t")
                        nc.tensor.matmul(
                            pa[:],
                            lhsT=yt01_t[:, tb * P : (tb + 1) * P],
                            rhs=w01[:, 0:QC],
                            start=True,
                            stop=False,
                        )
                    pb = pbs.get(tb)
                    if pb is None:
                        pb = pspb_pool.tile([P, DIM - QC], f32, name="pb_t")
                        nc.tensor.matmul(
                            pb[:],
                            lhsT=yt01_t[:, tb * P : (tb + 1) * P],
                            rhs=w01[:, QC:DIM],
                            start=True,
                            stop=False,
                        )
                    nc.tensor.matmul(
                        pa[:],
                        lhsT=yt2_t[:, tb * P : (tb + 1) * P],
                        rhs=w2[:, 0:QC],
                        start=False,
                        stop=True,
                    )
                    nc.tensor.matmul(
                        pb[:],
                        lhsT=yt2_t[:, tb * P : (tb + 1) * P],
                        rhs=w2[:, QC:DIM],
                        start=False,
                        stop=True,
                    )
                    ob_t = ob_pool.tile([P, DIM], f16, name="ob_t")
                    ce = (
                        nc.scalar.copy if tb % 2 == 0
                        else nc.vector.tensor_copy
                    )
                    ce(ob_t[:, 0:QC], pa[:])
                    ce(ob_t[:, QC:DIM], pb[:])
                    nc.sync.dma_start(
                        out[ds(j * QC + tb * P, P), :], ob_t[:]
                    )

            order = [(3, 0), (3, 1), (3, 2), (0, 0), (2, 0), (0, 1),
                     (2, 1), (0, 2), (2, 2), (1, 0), (1, 1), (1, 2)]
            # software pipeline: the PV of logits-tile i is emitted LAG
            # tiles after its QK/copy/exp, so the in-order PE queue always
            # has matmul work while the ACT engine runs the exps, and the
            # chunk-j projection trails one tile further still
            from collections import deque

            LAG = 8
            queue = deque()
            sub0 = {}  # first sub index of region (j, h)
            for n, (j, h, k) in enumerate(subs):
                if k == 0:
                    sub0[(j, h)] = n
            for j, h in order:
                issue_subs(sub0[(j, h)] + (j + 1) + 4, nc.gpsimd)
                qT_t = qT_ap(h, j)
                psy_box = [None]
                ntile = (j + 1) * 4
                for i in range(ntile):
                    c0 = max(0, P * i - QC * j)
                    psl_t = psl_pool.tile([P, QC], f32)
                    # QK first (can run before the bias arrives)
                    nc.tensor.matmul(
                        psl_t[:, c0:QC],
                        lhsT=kT_ap(h, i),
                        rhs=qT_t[:, c0:QC],
                        start=True,
                        stop=False,
                    )
                    # bias on top: fp8 DoubleRow identity copy (0.5 PE
                    # cycles/row); tile pairs are column-blocked in SBUF,
                    # the selector weights pick one tile per instruction
                    pr, sl = (i % 4) // 2, i % 2
                    nc.tensor.matmul(
                        psl_t[:, c0:QC],
                        lhsT=id8_t[
                            :, sl * 2 * P : (sl + 1) * 2 * P
                        ].rearrange("p (two m) -> p two m", two=2),
                        rhs=bias_ts[(j, h, i // 4)][
                            :, pr * 2 * QC : (pr + 1) * 2 * QC
                        ].rearrange("p (two q) -> p two q", two=2)[:, :, c0:],
                        start=False,
                        stop=True,
                        perf_mode=mybir.MatmulPerfMode.DoubleRow,
                    )
                    pe_t = pexp_pool.tile([P, QC], bf16)
                    nc.scalar.activation(
                        pe_t[:, c0:QC], psl_t[:, c0:QC], EXP
                    )

                    def mk_pv(j, h, i, c0, pe_t, psy_box):
                        def emit():
                            if i == 0:
                                psy_box[0] = psy_pool.tile(
                                    [P, QC], f32, name="psy_t"
                                )
                            psy_t = psy_box[0]
                            nc.tensor.matmul(
                                psy_t[:, c0:QC],
                                lhsT=va_ap(h, i),
                                rhs=pe_t[:, c0:QC],
                                start=(i == 0),
                                stop=(i == 4 * j + 3),
                            )
                            if i == 4 * j + 3:
                                if (j, h) == order[-1]:
                                    emit_final(j, psy_t)
                                    return
                                emit_drain(j, h, psy_t)
                                if h == 2:
                                    for tb in range(4):
                                        queue.append(
                                            mk_proj(
                                                j,
                                                tb,
                                                *yt_box[j],
                                                nc.vector.tensor_copy,
                                            )
                                        )
                        return emit

                    queue.append(mk_pv(j, h, i, c0, pe_t, psy_box))
                    while len(queue) > LAG:
                        queue.popleft()()
            while queue:
                queue.popleft()()

    nc.finalize()
    return nc


def _get_program():
    global _PROGRAM
    if _PROGRAM is None:
        _PROGRAM = _build_program()
    return _PROGRAM


def make_in_maps(q, k, v, attn_bias, W_proj):
    """Host-side sharding/layout prep: one input map per core."""
    q = np.asarray(q, dtype=np.float32)
    k = np.asarray(k, dtype=np.float32)
    v = np.asarray(v, dtype=np.float32)
    attn_bias = np.asarray(attn_bias, dtype=np.float32)
    W_proj = np.asarray(W_proj, dtype=np.float32)

    scale = 1.0 / math.sqrt(D)
    # causal mask in transposed [s, q] coords: masked where s > q
    smask = (np.arange(T)[:, None] > np.arange(T)[None, :]).astype(np.float32)
    smask *= -10000.0
    w_heads = W_proj.reshape(H, D, DIM)

    in_maps = []
    for c in range(NCORES):
        b = c // 4
        h0 = HPC * (c % 4)
        hs = slice(h0, h0 + HPC)
        qk = np.zeros((P + D, 2 * T), dtype=ml_dtypes.bfloat16)
        for hh in range(HPC):
            qk[hh * D : (hh + 1) * D, :T] = (
                q[b, h0 + hh].transpose(1, 0) * scale
            )
            qk[hh * D : (hh + 1) * D, T:] = k[b, h0 + hh].transpose(1, 0)
        va = np.zeros((HPC, P, NT, P), dtype=ml_dtypes.bfloat16)
        va[:, :, :, :D] = v[b, hs].reshape(HPC, NT, P, D).transpose(0, 2, 1, 3)
        va[:, :, :, D:] = 1.0
        biasT = attn_bias[b, hs].transpose(0, 2, 1) + smask[None]
        biasT = np.clip(biasT, -240.0, 240.0).astype(ml_dtypes.float8_e4m3)
        # region (j, h): s rows [0:(j+1)*512), q cols [j*512:(j+1)*512),
        # s-tile pairs column-interleaved for the DoubleRow copy
        regions = []
        for j, h in [(3, 0), (3, 1), (3, 2), (0, 0), (2, 0), (0, 1),
                     (2, 1), (0, 2), (2, 2), (1, 0), (1, 1), (1, 2)]:
            if True:
                r = biasT[h, 0 : (j + 1) * QC, j * QC : (j + 1) * QC]
                pairs = r.reshape((j + 1) * 2, 2, P, QC)
                regions.append(
                    pairs.transpose(0, 2, 1, 3).reshape((j + 1) * 2 * P, 2 * QC)
                )
        bias_pack = np.ascontiguousarray(np.concatenate(regions, axis=0))
        idp = np.zeros((P, 4 * P), dtype=ml_dtypes.float8_e4m3)
        eye = np.eye(P, dtype=np.float32)
        idp[:, 0:P] = eye        # selector A = [I | 0]
        idp[:, 3 * P :] = eye    # selector B = [0 | I]
        wp = np.zeros((P, 2 * DIM), dtype=ml_dtypes.bfloat16)
        wp[:D, :DIM] = w_heads[h0]
        wp[D:, :DIM] = w_heads[h0 + 1]
        wp[:D, DIM:] = w_heads[h0 + 2]
        in_maps.append(
            {
                "qk": qk,
                "va": va.reshape(HPC * P, T),
                "biasT": bias_pack,
                "idpk": idp,
                "wproj": wp,
            }
        )
    return in_maps


def assemble_output(results):
    """Sum the 4 per-core fp16 partial projections for each batch."""
    out = np.zeros((B, T, DIM), dtype=np.float32)
    for c in range(NCORES):
        out[c // 4] += results[c]["out"].astype(np.float32)
    return out


def kernel(q, k, v, attn_bias, W_proj):
    from concourse.bass_utils import run_bass_kernel_spmd

    nc = _get_program()
    in_maps = make_in_maps(q, k, v, attn_bias, W_proj)
    res = run_bass_kernel_spmd(nc, in_maps, list(range(NCORES)))
    return assemble_output(res.results)
